# revision 1
# baseline (speedup 1.0000x reference)
"""AdaptiveSpanAttention Trainium2 kernel (8 NeuronCores).

Sharding: core c -> (batch b = c//2, head-group g = c%2).
Each core computes, for its batch and its 8 heads:
  Q/K/V projections, anti-causal (j>=i) attention with adaptive-span
  mask, renormalization, and a partial output projection
  y_part = Out_g @ Wo[:, e_slice].T  (contraction over its 512 channels).
Host combines: y[b] = y_part[2b] + y_part[2b+1] + bo.

All matmuls in bf16 (f32 PSUM accumulation). Span-mask ramp in fp16
(exact for integer distances). No collectives.
"""
import sys

sys.path.insert(0, "/opt/trn_rl_repo")

from contextlib import ExitStack

import ml_dtypes
import numpy as np

import concourse.bass as bass
import concourse.tile as tile
from concourse import bacc, mybir
from concourse.bass_utils import run_bass_kernel_spmd

BF16 = mybir.dt.bfloat16
F16 = mybir.dt.float16
F32 = mybir.dt.float32

B, T, D, H = 4, 1024, 1024, 16
DH = 64          # head dim
R = 256.0
HC = 8           # heads per core
E = 512          # channels per core (HC * DH)
N_CORES = 8
TCH = 512        # t-chunk width (PSUM f32 free-dim limit)
NT = T // TCH    # 2 t-chunks
ST = T // 128    # 8 s-tiles
DT = D // 128    # 8 d-tiles

_NC_CACHE = {}

Z_MIN = 416.0  # verified on host per-call; span-mask restriction exact when z >= Z_MIN


def causal_width(st, tch):
    """Valid query-column width of block (s_tile=st, t_chunk=tch).

    Block covers s in [128*st, 128*st+128), t in [512*tch, 512*tch+512).
    Valid cells need s >= t, i.e. t' < delta + 128 with
    delta = 128*st - 512*tch.
    """
    delta = 128 * st - 512 * tch
    return max(0, min(TCH, delta + 128))


def span_width(st, tch, span_full):
    """Columns [0, m_w) where the span mask can differ from 1 (given z >= Z_MIN)."""
    delta = 128 * st - 512 * tch
    w = causal_width(st, tch)
    if span_full:
        return w
    return max(0, min(w, delta + 127 - int(Z_MIN)))


def build_nc(span_full=False):
    key = ("nc", span_full)
    if key in _NC_CACHE:
        return _NC_CACHE[key]
    nc = bacc.Bacc("TRN2", target_bir_lowering=False, debug=False, num_devices=1)

    # ---- DRAM parameters (per-core shards prepared on host) ----
    xT_d = nc.declare_dram_parameter("xT", [D, T], BF16, isOutput=False)
    WqT_d = nc.declare_dram_parameter("WqT", [D, E], BF16, isOutput=False)
    WkT_d = nc.declare_dram_parameter("WkT", [D, E], BF16, isOutput=False)
    WvT_d = nc.declare_dram_parameter("WvT", [D, E], BF16, isOutput=False)
    WoT_d = nc.declare_dram_parameter("WoT", [E, D], BF16, isOutput=False)
    WspT_d = nc.declare_dram_parameter("WspT", [D, HC], BF16, isOutput=False)
    bspan_d = nc.declare_dram_parameter("bspan", [1, HC], F32, isOutput=False)
    # packed span-ramp tiles: for each k with nonzero span width m_k, columns
    # [off_k, off_k+m_k) hold cneg[k, s', t'] = -(128k + s' - t')/R
    # (-60000 where causal-invalid)
    widths = [span_width(st, 0, span_full) for st in range(ST)]
    offs = np.concatenate([[0], np.cumsum(widths)]).astype(int)
    cneg_d = nc.declare_dram_parameter("cneg", [128, max(1, int(offs[-1]))],
                                       F16, isOutput=False)
    # c01[k, s', j] = 1.0 if s' >= j else 0.0  (causal 0/1 for t' = 128k + j)
    c01_d = nc.declare_dram_parameter("c01", [4, 128, 128], F16, isOutput=False)
    yp_d = nc.declare_dram_parameter("yp", [T, D], F32, isOutput=True)

    with tile.TileContext(nc) as tc, ExitStack() as ctx:
        # ---------------- pools ----------------
        consts = ctx.enter_context(tc.tile_pool(name="consts", bufs=1))
        xp = ctx.enter_context(tc.tile_pool(name="xp", bufs=1))
        wp = ctx.enter_context(tc.tile_pool(name="wp", bufs=1))
        qkp = ctx.enter_context(tc.tile_pool(name="qkp", bufs=1))
        vp = ctx.enter_context(tc.tile_pool(name="vp", bufs=1))
        outp = ctx.enter_context(tc.tile_pool(name="outp", bufs=1))
        scr = ctx.enter_context(tc.tile_pool(name="scr", bufs=3))
        ysb = ctx.enter_context(tc.tile_pool(name="ysb", bufs=5))

        ps_proj = ctx.enter_context(tc.tile_pool(name="ps_proj", bufs=2, space="PSUM"))
        lead_ctx = ExitStack()
        ps_lead = lead_ctx.enter_context(
            tc.tile_pool(name="ps_lead", bufs=6, space="PSUM"))

        # ---------------- critical-path loads: xT, Wq, Wk ----------------
        ones_row = consts.tile([1, 128], BF16)
        nc.vector.memset(ones_row[:], 1.0)

        xT_sb, wq_sb, wk_sb = [], [], []
        for dt_i in range(DT):
            t_ = xp.tile([128, T], BF16, tag="xT", bufs=DT, name=f"xT{dt_i}")
            nc.sync.dma_start(t_[:], xT_d[128 * dt_i:128 * (dt_i + 1), :])
            xT_sb.append(t_)
            t_ = wp.tile([128, E], BF16, tag="wq", bufs=DT, name=f"wq{dt_i}")
            nc.sync.dma_start(t_[:], WqT_d[128 * dt_i:128 * (dt_i + 1), :])
            wq_sb.append(t_)
        for dt_i in range(DT):
            t_ = wp.tile([128, E], BF16, tag="wk", bufs=DT, name=f"wk{dt_i}")
            nc.sync.dma_start(t_[:], WkT_d[128 * dt_i:128 * (dt_i + 1), :])
            wk_sb.append(t_)


        # span-net partial sums early: each reduce runs as its xT tile lands,
        # hidden under the DMA lead-in
        msum = consts.tile([128, DT], BF16)
        with nc.allow_low_precision(reason="span-net mean in bf16 is plenty"):
            for dt_i in range(DT):
                nc.vector.tensor_reduce(
                    msum[:, dt_i:dt_i + 1], xT_sb[dt_i][:],
                    mybir.AxisListType.X, mybir.AluOpType.add)

        # ---------------- Q/K projections (transposed layout) ----------------
        # QT[e, t] = sum_d WqT[d, e] * xT[d, t]; psum -> bf16 copies on ACT
        qt_sb = [qkp.tile([128, T], BF16, tag="qt", name=f"qt{i}", bufs=4)
                 for i in range(4)]
        kt_sb = [qkp.tile([128, T], BF16, tag="kt", name=f"kt{i}", bufs=4)
                 for i in range(4)]

        grp_ctr = [0]

        def emit_qtkt(et_list, copy_eng="act", pool=None):
            pool = pool or ps_proj
            for dst, w_sb in ((qt_sb, wq_sb), (kt_sb, wk_sb)):
                for et in et_list:
                    for tch in range(NT):
                        ps = pool.tile([128, TCH], F32, tag="pj",
                                       name=f"pj{et}_{tch}")
                        # rotate the contraction start so concurrent groups
                        # finish at different DMA-front positions
                        rot = 0
                        grp_ctr[0] += 1
                        order = [(rot + i) % DT for i in range(DT)]
                        for n_i, dt_i in enumerate(order):
                            nc.tensor.matmul(
                                ps[:],
                                w_sb[dt_i][:, 128 * et:128 * (et + 1)],
                                xT_sb[dt_i][:, TCH * tch:TCH * (tch + 1)],
                                start=(n_i == 0), stop=(n_i == DT - 1))
                        if copy_eng == "act":
                            nc.scalar.copy(
                                dst[et][:, TCH * tch:TCH * (tch + 1)], ps[:])
                        else:
                            nc.vector.tensor_copy(
                                dst[et][:, TCH * tch:TCH * (tch + 1)], ps[:])

        emit_qtkt([0], pool=ps_lead)

        def qtkt_fillers(et_list):
            """Per-(q/k, et, tch) psum-group closures for in-wave filling."""
            fs = []
            for dst, w_sb in ((qt_sb, wq_sb), (kt_sb, wk_sb)):
                for et in et_list:
                    for tch in range(NT):
                        def f(dst=dst, w_sb=w_sb, et=et, tch=tch):
                            ps = ps_proj.tile([128, TCH], F32, tag="pj",
                                              name=f"pjf{et}_{tch}")
                            for dt_i in range(DT):
                                nc.tensor.matmul(
                                    ps[:],
                                    w_sb[dt_i][:, 128 * et:128 * (et + 1)],
                                    xT_sb[dt_i][:, TCH * tch:TCH * (tch + 1)],
                                    start=(dt_i == 0), stop=(dt_i == DT - 1))
                            nc.scalar.copy(
                                dst[et][:, TCH * tch:TCH * (tch + 1)], ps[:])
                        fs.append(f)
            return fs

        # ---------------- V (natural layout, ones-augmented) ----------------
        # v_aug[st][p, h, 0:64] = V[128*st+p, 64h+j]; v_aug[st][p, h, 64:128] = 1
        # (64 ones columns make attn@V produce the denominator W broadcast
        #  across psum partitions 64:128)
        wv_sb = []
        for dt_i in range(DT):
            t_ = wp.tile([128, E], BF16, tag="wv", bufs=DT, name=f"wv{dt_i}")
            nc.sync.dma_start(t_[:], WvT_d[128 * dt_i:128 * (dt_i + 1), :])
            wv_sb.append(t_)

        v_aug = [None] * ST

        def emit_v(st, pool=None):
            pool = pool or ps_proj
            va = vp.tile([128, HC, 2 * DH], BF16, tag="vaug", bufs=ST,
                         name=f"vaug{st}")
            nc.gpsimd.memset(va[:, :, DH:2 * DH], 1.0)
            ps = pool.tile([128, E], F32, tag="pj", name=f"pjv{st}")
            rot = 0
            grp_ctr[0] += 1
            order = [(rot + i) % DT for i in range(DT)]
            for n_i, dt_i in enumerate(order):
                nc.tensor.matmul(
                    ps[:],
                    xT_sb[dt_i][:, 128 * st:128 * (st + 1)],
                    wv_sb[dt_i][:],
                    start=(n_i == 0), stop=(n_i == DT - 1))
            nc.vector.tensor_copy(
                va[:, :, 0:DH], ps[:].rearrange("p (h d) -> p h d", h=HC))
            v_aug[st] = va

        emit_v(0, pool=ps_lead)
        emit_v(1, pool=ps_lead)
        emit_v(2, pool=ps_lead)
        emit_v(3, pool=ps_lead)
        emit_qtkt([1], pool=ps_lead)

        # ---------------- remaining loads ----------------
        bspan_sb = consts.tile([1, HC], F32)
        nc.sync.dma_start(bspan_sb[:], bspan_d[:, :])
        wsp_sb = []
        for dt_i in range(DT):
            t_ = wp.tile([128, HC], BF16, tag="wsp", bufs=DT, name=f"wsp{dt_i}")
            nc.sync.dma_start(t_[:], WspT_d[128 * dt_i:128 * (dt_i + 1), :])
            wsp_sb.append(t_)
        wo_sb = []
        for j in range(4):
            t_ = wp.tile([128, D], BF16, tag="wo", bufs=4, name=f"wo{j}")
            nc.sync.dma_start(t_[:], WoT_d[128 * j:128 * (j + 1), :])
            wo_sb.append(t_)
        cneg_sb = consts.tile([128, max(1, int(offs[-1]))], F16, tag="cneg")
        nc.sync.dma_start(cneg_sb[:], cneg_d[:, :])
        c01_sb = []
        for k in range(4):
            ct2 = consts.tile([128, 128], F16, tag="c01", bufs=4,
                              name=f"c01_{k}")
            nc.sync.dma_start(ct2[:], c01_d[k])
            c01_sb.append(ct2)

        # ---------------- span net ----------------
        # logit = (sum_t x)/T @ WspanT + bspan; a = 1 + (T/R)*sigmoid(logit)
        zlog = ps_lead.tile([1, HC], F32, tag="pj", padded_shape=[128, TCH])
        for dt_i in range(DT):
            nc.tensor.matmul(zlog[:], msum[:, dt_i:dt_i + 1], wsp_sb[dt_i][:],
                             start=(dt_i == 0), stop=(dt_i == DT - 1))
        zrow = consts.tile([1, HC], F32)
        nc.vector.scalar_tensor_tensor(
            zrow[:], zlog[:], 1.0 / T, bspan_sb[:],
            op0=mybir.AluOpType.mult, op1=mybir.AluOpType.add)
        # sigmoid via exp so ACT never leaves the exp_and_others LUT table
        # (a Sigmoid call would cost two 1283 ns table switches mid-stream)
        en = consts.tile([1, HC], F32)
        nc.scalar.activation(en[:], zrow[:],
                             mybir.ActivationFunctionType.Exp, scale=-1.0)
        den = consts.tile([1, HC], F32)
        nc.vector.tensor_scalar_add(den[:], en[:], 1.0)
        sig = consts.tile([1, HC], BF16)
        with nc.allow_low_precision(reason="span sigmoid recip in bf16"):
            nc.vector.reciprocal(sig[:], den[:])
        a_ps = ps_lead.tile([128, HC], F32, tag="pj", padded_shape=[128, TCH])
        nc.tensor.matmul(a_ps[:], ones_row[:], sig[:], start=True, stop=True)
        a_sb = consts.tile([128, HC], F32)
        nc.scalar.activation(a_sb[:], a_ps[:],
                             mybir.ActivationFunctionType.Identity,
                             scale=T / R, bias=1.0)

        lead_ctx.close()
        ps_sc = ctx.enter_context(tc.tile_pool(name="ps_sc", bufs=2, space="PSUM"))
        ps_out = ctx.enter_context(tc.tile_pool(name="ps_out", bufs=2, space="PSUM"))

        # ---------------- attention ----------------
        # out_pair[j][tch] holds heads 2j (parts 0:64) and 2j+1 (parts 64:128)
        out_pair = [[outp.tile([128, TCH], BF16, tag="out", bufs=8,
                               name=f"op{j}_{c}") for c in range(NT)]
                    for j in range(4)]

        def attn_pair(tch, j, v_prefetch=False, fillers=()):
            """Attention for head pair (2j, 2j+1); both share et=j.

            Scores for the two heads go into one 2-bank psum pair-tile so a
            single exp covers both; even/odd heads sit at partition bases
            0/64 so their K=64 score matmuls row-pack on silicon.
            """
            first_st = 4 * tch
            heads = (2 * j, 2 * j + 1)
            pouts = [ps_out.tile([128, TCH], F32, tag="pout",
                                 name=f"pout{h}_{tch}") for h in heads]
            fillers = list(fillers)
            for st in range(first_st, ST):
                if v_prefetch and st + 4 < ST and v_aug[st + 4] is None:
                    emit_v(st + 4)
                if fillers:
                    fillers.pop(0)()
                w = causal_width(st, tch)
                m_w = span_width(st, tch, span_full)
                k = st - first_st  # delta = 128*k
                sc_hp = ps_sc.tile([128, 2, TCH], F32, tag="sc",
                                   name=f"sc{j}_{st}")
                for i, h in enumerate(heads):
                    hp = (h % 2) * 64
                    nc.tensor.matmul(
                        sc_hp[:, i, 0:w],
                        kt_sb[j][hp:hp + DH, 128 * st:128 * (st + 1)],
                        qt_sb[j][hp:hp + DH, TCH * tch:TCH * tch + w],
                        start=True, stop=True)
                p_hp = scr.tile([128, 2, TCH], BF16, tag="p", bufs=10,
                                name=f"p{j}_{st}")
                nc.scalar.activation(
                    p_hp[:, :, 0:w], sc_hp[:, :, 0:w],
                    mybir.ActivationFunctionType.Exp, scale=1.0 / 8.0)
                for i, h in enumerate(heads):
                    if k <= 3:
                        # diagonal block: causal zeroing on t' in [128k, w)
                        d0 = 128 * k
                        ceng = nc.vector if tch == 0 else nc.gpsimd
                        ceng.tensor_mul(
                            p_hp[:, i, d0:w], p_hp[:, i, d0:w],
                            c01_sb[k][:, 0:w - d0])
                    if m_w > 0:
                        # span mask: pm = min(max(a_h + cneg, 0), 1) * p
                        mt = scr.tile([128, TCH], F16, tag="mt", bufs=10,
                                      name=f"mt{h}_{st}")
                        nc.vector.tensor_scalar(
                            mt[:, 0:m_w],
                            cneg_sb[:, offs[k]:offs[k] + m_w],
                            a_sb[:, h:h + 1], 0.0,
                            op0=mybir.AluOpType.add, op1=mybir.AluOpType.max)
                        nc.vector.scalar_tensor_tensor(
                            p_hp[:, i, 0:m_w], mt[:, 0:m_w], 1.0,
                            p_hp[:, i, 0:m_w],
                            op0=mybir.AluOpType.min, op1=mybir.AluOpType.mult)
                for i, h in enumerate(heads):
                    nc.tensor.matmul(
                        pouts[i][:, 0:w], v_aug[st][:, h, :],
                        p_hp[:, i, 0:w],
                        start=(st == first_st), stop=(st == ST - 1),
                        skip_group_check=True)
            for i, h in enumerate(heads):
                # rows 0:64 numerator; rows 64:128 denominator W (broadcast)
                hp = (h % 2) * 64
                pout = pouts[i]
                rw = scr.tile([DH, TCH], F32, tag="rw", bufs=8,
                              name=f"rw{h}")
                with nc.allow_low_precision(reason="denom recip bf16"):
                    nc.vector.reciprocal(rw[:], pout[DH:2 * DH, :])
                nc.vector.scalar_tensor_tensor(
                    out_pair[j][tch][hp:hp + DH, :], pout[0:DH, :], 1.0,
                    rw[:],
                    op0=mybir.AluOpType.mult, op1=mybir.AluOpType.mult)

        def out_proj(tch, tts=None):
            for tt in (tts if tts is not None else range(4 * tch, 4 * (tch + 1))):
                toff = 128 * tt - TCH * tch
                for nch in range(NT):
                    yps = ps_proj.tile([128, TCH], F32, tag="pj",
                                       name=f"y{tt}_{nch}")
                    for j in range(4):
                        nc.tensor.matmul(
                            yps[:],
                            out_pair[j][tch][:, toff:toff + 128],
                            wo_sb[j][:, TCH * nch:TCH * (nch + 1)],
                            start=(j == 0), stop=(j == 3))
                    yo = ysb.tile([128, TCH], F32, tag="y")
                    if tch == 0:
                        nc.vector.tensor_copy(yo[:], yps[:])
                    else:
                        nc.scalar.copy(yo[:], yps[:])
                    nc.sync.dma_start(
                        yp_d[128 * tt:128 * (tt + 1),
                             TCH * nch:TCH * (nch + 1)],
                        yo[:])

        attn_pair(0, 0, v_prefetch=True)
        attn_pair(0, 1)
        emit_qtkt([2])
        attn_pair(0, 2)
        emit_qtkt([3])
        attn_pair(0, 3)
        out_proj(0, tts=[0])
        attn_pair(1, 0)
        out_proj(0, tts=[1])
        attn_pair(1, 1)
        out_proj(0, tts=[2])
        attn_pair(1, 2)
        out_proj(0, tts=[3])
        attn_pair(1, 3)
        out_proj(1)

    nc.compile()
    _NC_CACHE[key] = nc
    return nc


def _prep_core_inputs(x, Wq, Wk, Wv, Wo, Wspan, bspan, cneg, c01):
    bf = ml_dtypes.bfloat16
    in_maps = []
    for c in range(N_CORES):
        b, g = c // 2, c % 2
        hs = slice(E * g, E * (g + 1))
        in_maps.append({
            "c01": c01,
            "xT": np.ascontiguousarray(x[b].T).astype(bf),
            "WqT": np.ascontiguousarray(Wq[hs, :].T).astype(bf),
            "WkT": np.ascontiguousarray(Wk[hs, :].T).astype(bf),
            "WvT": np.ascontiguousarray(Wv[hs, :].T).astype(bf),
            "WoT": np.ascontiguousarray(Wo[:, hs].T).astype(bf),
            "WspT": np.ascontiguousarray(Wspan[HC * g:HC * (g + 1), :].T).astype(bf),
            "bspan": np.asarray(bspan[HC * g:HC * (g + 1)], np.float32).reshape(1, HC),
            "cneg": cneg,
        })
    return in_maps


def _make_c01():
    sp = np.arange(128, dtype=np.float32)[:, None]
    jp = np.arange(128, dtype=np.float32)[None, :]
    return np.stack([(sp - jp >= 0) for _ in range(4)]).astype(np.float16)


def _make_cneg(span_full):
    sp = np.arange(128, dtype=np.float32)[:, None]
    cols = []
    for k in range(ST):
        m_w = span_width(k, 0, span_full)
        if m_w == 0:
            continue
        tp = np.arange(m_w, dtype=np.float32)[None, :]
        d = 128.0 * k + sp - tp
        ramp = np.where(d < 0, -60000.0, -d / R)
        cols.append(ramp)
    if not cols:
        return np.zeros((128, 1), np.float16)
    return np.concatenate(cols, axis=1).astype(np.float16)


def kernel(x, Wq, Wk, Wv, Wo, bo, Wspan, bspan):
    x = np.asarray(x, np.float32)
    Wq = np.asarray(Wq, np.float32)
    Wk = np.asarray(Wk, np.float32)
    Wv = np.asarray(Wv, np.float32)
    Wo = np.asarray(Wo, np.float32)
    bo = np.asarray(bo, np.float32)
    Wspan = np.asarray(Wspan, np.float32)
    bspan = np.asarray(bspan, np.float32)

    # span-mask restriction is only exact when every z >= Z_MIN; verify on host
    logits = x.mean(axis=1) @ Wspan.T + bspan
    z = T / (1.0 + np.exp(-logits))
    span_full = bool(z.min() < Z_MIN + 8.0)
    nc = build_nc(span_full=span_full)
    in_maps = _prep_core_inputs(x, Wq, Wk, Wv, Wo, Wspan, bspan,
                                _make_cneg(span_full), _make_c01())
    res = run_bass_kernel_spmd(nc, in_maps, core_ids=list(range(N_CORES)))
    y = np.empty((B, T, D), np.float32)
    for b in range(B):
        y[b] = res.results[2 * b]["yp"] + res.results[2 * b + 1]["yp"] + bo
    return y



# revision 38
# speedup vs baseline: 1.1314x; 1.1314x over previous
"""AdaptiveSpanAttention Trainium2 kernel (8 NeuronCores).

Sharding: core c -> (batch b = c//2, head-group g = c%2).
Each core computes, for its batch and its 8 heads:
  Q/K/V projections in error-compensated fp8 DoubleRow (x and W split
  into fp8e4 hi+lo on host; the 3 significant cross products run with
  pair-slots packing two 128-k-tiles per pass -> 0.75 cycles/row vs
  bf16), anti-causal (j>=i) attention with adaptive-span mask in bf16,
  renormalization, and a partial bf16 output projection
  y_part = Out_g @ Wo[:, e_slice].T.
Host combines: y[b] = yp[2b] + yp[2b+1] + bo  (yp emitted as f16).

The span net (z = T*sigmoid(mean_t x @ WspanT + bspan)) is computed on
host in f32 and shipped as the per-head ramp offset a = 1 + z/R.

Causal zeroing of diagonal blocks is folded into the score PSUM as a
-1e9 bias added by an identity-weight matmul before the exp, so the
exp -> attnV chain has no extra vector-engine stage.

Projection weights are pre-scaled by SW=128 on host so the fp8 lo
residuals stay in e4m3's normal range; the inverse scales fold into
the exp scale and the renorm multiplier.

DMAs are batched (one per tensor, partition-major host layout) because
the cost of a DMA is dominated by a serialized ~650ns issue slot.
"""
import sys

sys.path.insert(0, "/opt/trn_rl_repo")

from contextlib import ExitStack

import ml_dtypes
import numpy as np

import concourse.bass as bass
import concourse.tile as tile
from concourse import bacc, mybir
from concourse.bass_utils import run_bass_kernel_spmd

BF16 = mybir.dt.bfloat16
F16 = mybir.dt.float16
FP8 = mybir.dt.float8e4
F32 = mybir.dt.float32
DR = mybir.MatmulPerfMode.DoubleRow

B, T, D, H = 4, 1024, 1024, 16
DH = 64          # head dim
R = 256.0
HC = 8           # heads per core
E = 512          # channels per core (HC * DH)
N_CORES = 8
TCH = 512        # t-chunk width (PSUM f32 free-dim limit)
NT = T // TCH    # 2 t-chunks
ST = T // 128    # 8 s-tiles
NM = 4           # contraction pair-tiles (1024 = 4 * 256)

SW = 128.0       # host pre-scale on projection weights
OS = 8.0         # attn-out pre-scale before its fp8 hi/lo split
EXP_SCALE = 1.0 / (8.0 * SW * SW)   # folds 1/sqrt(dh) and Q/K weight scales
Y_SCALE = 1.0 / (SW * OS)           # folds Wo and attn-out scales back out

_NC_CACHE = {}

# span-mask restriction bounds, verified on host per call (span_full
# fallback otherwise). z in [Z_MIN+6, Z_MAX-6] required.
Z_MIN = 490.0
Z_MAX = 545.0
CUT = int(R + Z_MAX)  # distance beyond which attention is exactly 0


def causal_width(st, tch):
    """Valid query-column width of block (s_tile=st, t_chunk=tch)."""
    delta = 128 * st - 512 * tch
    return max(0, min(TCH, delta + 128))


def span_width(st, tch, span_full):
    """Columns [0, m_w) where the span mask can differ from 1 (z >= Z_MIN)."""
    delta = 128 * st - 512 * tch
    w = causal_width(st, tch)
    if span_full:
        return w
    return max(0, min(w, delta + 127 - int(Z_MIN)))


def t_lo(st, tch, span_full):
    """Columns [0, t_lo) of the block are fully masked (dist >= R + z)."""
    if span_full:
        return 0
    delta = 128 * st - 512 * tch
    return max(0, delta - CUT)


def build_nc(span_full=False):
    key = ("nc", span_full)
    if key in _NC_CACHE:
        return _NC_CACHE[key]
    nc = bacc.Bacc("TRN2", target_bir_lowering=False, debug=False, num_devices=1)

    # ---- DRAM parameters (per-core shards, partition-major batched) ----
    # x pair tiles: [128 part][NM][2 slots][T]; slot i of pair tile m holds
    # xT rows [256m+128i, 256m+128(i+1)). Split into two halves (m 0-1, 2-3)
    # so the PE can start before the whole tensor lands.
    xh0_d = nc.declare_dram_parameter("xh0", [128, 2, 2, T], FP8, isOutput=False)
    xh1_d = nc.declare_dram_parameter("xh1", [128, 2, 2, T], FP8, isOutput=False)
    xl0_d = nc.declare_dram_parameter("xl0", [128, 2, 2, T], FP8, isOutput=False)
    xl1_d = nc.declare_dram_parameter("xl1", [128, 2, 2, T], FP8, isOutput=False)
    w_d = {}
    for wname in ("wq", "wk", "wv"):
        for lv in ("h", "l"):
            w_d[wname + lv] = nc.declare_dram_parameter(
                wname + lv, [128, NM, 2, E], FP8, isOutput=False)
    woT_d = nc.declare_dram_parameter("woT", [128, 4, D], BF16, isOutput=False)
    arow_d = nc.declare_dram_parameter("arow", [1, HC], F16, isOutput=False)
    # packed span-ramp tiles (see _make_cneg); widths account for the
    # fully-masked column cut
    widths = [max(0, span_width(st, tc, span_full) - t_lo(st, tc, span_full))
              for tc in range(NT) for st in range(ST)]
    offs = np.concatenate([[0], np.cumsum(widths)]).astype(int)
    cneg_d = nc.declare_dram_parameter("cneg", [128, max(1, int(offs[-1]))],
                                       F16, isOutput=False)
    # cmask[:, k] for k<4: 0 where s' >= j else -1e9 (causal bias);
    # cmask[:, 4] = identity (weights for the bias matmul)
    cmask_d = nc.declare_dram_parameter("cmask", [128, 5, 128], BF16,
                                        isOutput=False)
    yp_d = nc.declare_dram_parameter("yp", [T, D], F16, isOutput=True)

    with tile.TileContext(nc) as tc, ExitStack() as ctx:
        # ---------------- pools ----------------
        consts = ctx.enter_context(tc.tile_pool(name="consts", bufs=1))
        xp = ctx.enter_context(tc.tile_pool(name="xp", bufs=1))
        wp = ctx.enter_context(tc.tile_pool(name="wp", bufs=1))
        qkp = ctx.enter_context(tc.tile_pool(name="qkp", bufs=1))
        vp = ctx.enter_context(tc.tile_pool(name="vp", bufs=1))
        outp = ctx.enter_context(tc.tile_pool(name="outp", bufs=1))
        scr = ctx.enter_context(tc.tile_pool(name="scr", bufs=3))
        ysb = ctx.enter_context(tc.tile_pool(name="ysb", bufs=3))

        lead_ctx = ExitStack()
        ps_lead = lead_ctx.enter_context(
            tc.tile_pool(name="ps_lead", bufs=7, space="PSUM"))
        ps_warm = lead_ctx.enter_context(
            tc.tile_pool(name="ps_warm", bufs=1, space="PSUM"))

        # ---------------- PE p-state warmup ----------------
        # The PE clock ramps with sustained use and resets on idle gaps.
        # Dummy matmuls on a zeroed tile keep it hot through the DMA lead-in.
        warm = consts.tile([128, TCH], BF16)
        nc.gpsimd.memset(warm[:], 0.0)
        wps_holder = [None]

        def dummy(n=1, ap=TCH):
            if wps_holder[0] is None:
                wps_holder[0] = ps_warm.tile([128, TCH], F32, tag="warm",
                                             name="warmps")
            for _ in range(n):
                nc.tensor.matmul(wps_holder[0][:, 0:ap], warm[:, 0:128],
                                 warm[:, 0:ap], start=True, stop=True)

        dummy(6)

        # ---------------- batched DMA loads ----------------
        ones_row = consts.tile([1, 128], F16)
        nc.vector.memset(ones_row[:], 1.0)

        xh_sb = xp.tile([128, 2, 2, 2, T], FP8, name="xh_sb")
        xl_sb = xp.tile([128, 2, 2, 2, T], FP8, name="xl_sb")
        nc.sync.dma_start(xh_sb[:, 0], xh0_d[:, :, :, :])
        wq_h = wp.tile([128, NM, 2, E], FP8, name="wq_h")
        nc.sync.dma_start(wq_h[:], w_d["wqh"][:, :, :, :])
        nc.sync.dma_start(xh_sb[:, 1], xh1_d[:, :, :, :])
        wk_h = wp.tile([128, NM, 2, E], FP8, name="wk_h")
        nc.sync.dma_start(wk_h[:], w_d["wkh"][:, :, :, :])
        nc.sync.dma_start(xl_sb[:, 0], xl0_d[:, :, :, :])
        wq_l = wp.tile([128, NM, 2, E], FP8, name="wq_l")
        nc.sync.dma_start(wq_l[:], w_d["wql"][:, :, :, :])
        nc.sync.dma_start(xl_sb[:, 1], xl1_d[:, :, :, :])
        wk_l = wp.tile([128, NM, 2, E], FP8, name="wk_l")
        nc.sync.dma_start(wk_l[:], w_d["wkl"][:, :, :, :])
        arow_sb = consts.tile([1, HC], F16)
        nc.sync.dma_start(arow_sb[:], arow_d[:, :])
        cneg_sb = consts.tile([128, max(1, int(offs[-1]))], F16, tag="cneg")
        nc.sync.dma_start(cneg_sb[:], cneg_d[:, :])
        cmask_sb = consts.tile([128, 5, 128], BF16, name="cmask_sb")
        nc.sync.dma_start(cmask_sb[:], cmask_d[:, :, :])
        wv_h = wp.tile([128, NM, 2, E], FP8, name="wv_h")
        nc.sync.dma_start(wv_h[:], w_d["wvh"][:, :, :, :])
        wv_l = wp.tile([128, NM, 2, E], FP8, name="wv_l")
        nc.sync.dma_start(wv_l[:], w_d["wvl"][:, :, :, :])
        wo_sb = wp.tile([128, 4, D], BF16, name="wo_sb")
        nc.sync.dma_start(wo_sb[:], woT_d[:, :, :])

        def xm(hi, m):
            t_ = xh_sb if hi else xl_sb
            return t_[:, m // 2, m % 2]

        wsb = {"wqh": wq_h, "wql": wq_l, "wkh": wk_h, "wkl": wk_l,
               "wvh": wv_h, "wvl": wv_l}

        # ---------------- Q/K projections (transposed layout) ----------------
        # QT[e, t] = sum_d WqT'[d, e] * xT[d, t] in compensated fp8.
        # Per pair-tile m the 3 products (hi.hi, lo_w.hi_x, hi_w.lo_x) run as
        # DoubleRow passes; lead groups are emitted m-major so the PE chases
        # the DMA stream.
        qt_sb = [qkp.tile([128, T], BF16, tag="qt", name=f"qt{i}", bufs=4)
                 for i in range(4)]
        kt_sb = [qkp.tile([128, T], BF16, tag="kt", name=f"kt{i}", bufs=4)
                 for i in range(4)]

        def qk_mm(ps, wn, et, tch, m, prod, first=False, last=False):
            """One product matmul: prod 0 = hi.hi, 1 = hi_w.lo_x,
            2 = lo_w.hi_x."""
            es = slice(128 * et, 128 * (et + 1))
            ts = slice(TCH * tch, TCH * (tch + 1))
            w_t = wsb[wn + ("h" if prod < 2 else "l")][:, m]
            x_t = xm(1 if prod != 1 else 0, m)
            nc.tensor.matmul(
                ps[:], w_t[:, :, es], x_t[:, :, ts],
                start=first, stop=last, perf_mode=DR, skip_group_check=True)

        def qk_copy(dst_sb, et, tch, ps):
            ts = slice(TCH * tch, TCH * (tch + 1))
            nc.scalar.copy(dst_sb[et][:, ts], ps[:])

        # 7 lead groups chase the DMA stream in availability order:
        # all hi.hi products (x_hi + W_hi land first), then hi_w.lo_x
        # (x_lo next), then lo_w.hi_x (W_lo last)
        lead_defs = [
            (qt_sb, "wq", 0, 0), (kt_sb, "wk", 0, 0),
            (qt_sb, "wq", 1, 0), (kt_sb, "wk", 1, 0),
            (qt_sb, "wq", 0, 1), (kt_sb, "wk", 0, 1),
            (qt_sb, "wq", 1, 1),
        ]
        lead_ps = [ps_lead.tile([128, TCH], F32, tag="pj", name=f"pl{i}")
                   for i in range(len(lead_defs))]
        for prod in range(3):
            for m in range(NM):
                for gi, (dst, wn, et, tch) in enumerate(lead_defs):
                    qk_mm(lead_ps[gi], wn, et, tch, m, prod,
                          first=(prod == 0 and m == 0),
                          last=(prod == 2 and m == NM - 1))
                if prod == 0:
                    dummy(2)
                elif prod == 1 and m < 2:
                    dummy(1)
        for gi, (dst, wn, et, tch) in enumerate(lead_defs):
            qk_copy(dst, et, tch, lead_ps[gi])

        def emit_qk(dst_sb, wn, et, tch, pool):
            ps = pool.tile([128, TCH], F32, tag="pj", name=f"pj{et}_{tch}")
            for m in range(NM):
                for prod in range(3):
                    qk_mm(ps, wn, et, tch, m, prod,
                          first=(m == 0 and prod == 0),
                          last=(m == NM - 1 and prod == 2))
            qk_copy(dst_sb, et, tch, ps)

        # ---------------- V (natural layout, ones-augmented) ----------------
        v_aug = [None] * ST

        def emit_v(st, pool):
            va = vp.tile([128, HC, 2 * DH], BF16, tag="vaug", bufs=ST,
                         name=f"vaug{st}")
            nc.gpsimd.memset(va[:, :, DH:2 * DH], 1.0)
            ps = pool.tile([128, E], F32, tag="pj", name=f"pjv{st}",
                           padded_shape=[128, TCH])
            ss = slice(128 * st, 128 * (st + 1))
            ops = ([(xm(1, m), wv_h[:, m]) for m in range(NM)]
                   + [(xm(1, m), wv_l[:, m]) for m in range(NM)]
                   + [(xm(0, m), wv_h[:, m]) for m in range(NM)])
            for i, (x_t, w_t) in enumerate(ops):
                nc.tensor.matmul(
                    ps[:], x_t[:, :, ss], w_t[:],
                    start=(i == 0), stop=(i == len(ops) - 1), perf_mode=DR,
                    skip_group_check=True)
            nc.vector.tensor_copy(
                va[:, :, 0:DH], ps[:].rearrange("p (h d) -> p h d", h=HC))
            v_aug[st] = va

        emit_qk(kt_sb, "wk", 1, 1, ps_lead)
        emit_v(0, ps_lead)
        emit_v(1, ps_lead)
        emit_v(2, ps_lead)
        emit_v(3, ps_lead)

        # span ramp offset (host-computed): a = 1 + z/R -> [128, HC]
        a_ps = ps_lead.tile([128, HC], F32, tag="pj", padded_shape=[128, TCH])
        nc.tensor.matmul(a_ps[:], ones_row[:], arow_sb[:], start=True,
                         stop=True)
        a_sb = consts.tile([128, HC], F32)
        nc.vector.tensor_copy(a_sb[:], a_ps[:])

        lead_ctx.close()
        ps_proj = ctx.enter_context(tc.tile_pool(name="ps_proj", bufs=2, space="PSUM"))
        ps_sc = ctx.enter_context(tc.tile_pool(name="ps_sc", bufs=2, space="PSUM"))
        ps_out = ctx.enter_context(tc.tile_pool(name="ps_out", bufs=2, space="PSUM"))

        # ---------------- attention ----------------
        # out_pair[j][tch] holds heads 2j (parts 0:64) and 2j+1 (parts 64:128)
        out_pair = [[outp.tile([128, TCH], BF16, tag="out", bufs=8,
                               name=f"op{j}_{c}") for c in range(NT)]
                    for j in range(4)]

        def attn_pair(tch, j, v_prefetch=False, fillers=(), renorm_tts=1):
            """Attention for head pair (2j, 2j+1); both share et=j."""
            first_st = 4 * tch
            heads = (2 * j, 2 * j + 1)
            pouts = [ps_out.tile([128, TCH], F32, tag="pout",
                                 name=f"pout{h}_{tch}") for h in heads]
            fillers = list(fillers)
            for st in range(first_st, ST):
                if v_prefetch and st + 4 < ST and v_aug[st + 4] is None:
                    emit_v(st + 4, ps_proj)
                if fillers:
                    f = fillers.pop(0)
                    if f is not None:
                        f()
                w = causal_width(st, tch)
                k = st - first_st  # delta = 128*k
                tlo = t_lo(st, tch, span_full)
                m_w = span_width(st, tch, span_full)
                moff = offs[8 * tch + st]
                sc_hp = ps_sc.tile([128, 2, TCH], F32, tag="sc",
                                   name=f"sc{j}_{st}")
                diag = k <= 3
                for i, h in enumerate(heads):
                    hp = (h % 2) * 64
                    nc.tensor.matmul(
                        sc_hp[:, i, tlo:w],
                        kt_sb[j][hp:hp + DH, 128 * st:128 * (st + 1)],
                        qt_sb[j][hp:hp + DH, TCH * tch + tlo:TCH * tch + w],
                        start=True, stop=not diag, skip_group_check=True)
                if diag:
                    # causal bias: identity-weight matmul adds -1e9 where
                    # j > s' on the diagonal 128x128 sub-block, pre-exp
                    d0 = 128 * k
                    for i in range(2):
                        nc.tensor.matmul(
                            sc_hp[:, i, d0:w], cmask_sb[:, 4],
                            cmask_sb[:, k, 0:w - d0],
                            start=False, stop=(i == 1),
                            skip_group_check=True)
                p_hp = scr.tile([128, 2, TCH], BF16, tag="p", bufs=10,
                                name=f"p{j}_{st}")
                nc.scalar.activation(
                    p_hp[:, :, tlo:w], sc_hp[:, :, tlo:w],
                    mybir.ActivationFunctionType.Exp, scale=EXP_SCALE)
                if m_w > tlo:
                    for i, h in enumerate(heads):
                        # span mask: pm = min(max(a_h + cneg, 0), 1) * p
                        mt = scr.tile([128, TCH], F16, tag="mt", bufs=10,
                                      name=f"mt{h}_{st}")
                        nc.vector.tensor_scalar(
                            mt[:, 0:m_w - tlo],
                            cneg_sb[:, moff:moff + m_w - tlo],
                            a_sb[:, h:h + 1], 0.0,
                            op0=mybir.AluOpType.add, op1=mybir.AluOpType.max)
                        nc.vector.scalar_tensor_tensor(
                            p_hp[:, i, tlo:m_w], mt[:, 0:m_w - tlo], 1.0,
                            p_hp[:, i, tlo:m_w],
                            op0=mybir.AluOpType.min, op1=mybir.AluOpType.mult)
                for i, h in enumerate(heads):
                    nc.tensor.matmul(
                        pouts[i][:, tlo:w], v_aug[st][:, h, :],
                        p_hp[:, i, tlo:w],
                        start=(st == first_st), stop=(st == ST - 1),
                        skip_group_check=True)
            # rows 0:64 numerator (scaled SW); rows 64:128 denominator W.
            # Copy each pout to SBUF first (ACT) so the PSUM bank frees
            # ~2us earlier -- the next pair's first attnV matmul reuses it.
            pout_sb = []
            for i, h in enumerate(heads):
                po = scr.tile([128, TCH], F32, tag="posb", bufs=4,
                              name=f"posb{h}")
                nc.scalar.copy(po[:], pouts[i][:])
                pout_sb.append(po)
            nchunk = TCH // renorm_tts
            for rchunk in range(renorm_tts):
                cs = slice(rchunk * nchunk, (rchunk + 1) * nchunk)
                for i, h in enumerate(heads):
                    hp = (h % 2) * 64
                    pout = pout_sb[i]
                    rw = scr.tile([DH, TCH], F32, tag="rw", bufs=8,
                                  name=f"rw{h}_{rchunk}")
                    with nc.allow_low_precision(reason="denom recip"):
                        nc.vector.reciprocal(rw[:, cs], pout[DH:2 * DH, cs])
                    nc.vector.scalar_tensor_tensor(
                        out_pair[j][tch][hp:hp + DH, cs], pout[0:DH, cs],
                        1.0 / SW, rw[:, cs],
                        op0=mybir.AluOpType.mult, op1=mybir.AluOpType.mult)

        def op_matmuls(yps, tch, toff, ns):
            for j in range(4):
                nc.tensor.matmul(
                    yps[:], out_pair[j][tch][:, toff:toff + 128],
                    wo_sb[:, j, ns],
                    start=(j == 0), stop=(j == 3), skip_group_check=True)

        def yo_copy(yo, ns, yps, eng):
            with nc.allow_low_precision(reason="f16 output"):
                if eng % 2 == 0:
                    nc.scalar.copy(yo[:, ns], yps[:])
                else:
                    nc.vector.tensor_copy(yo[:, ns], yps[:])

        def op_tt(tt, split_dma=False):
            """Out-projection for t-tile tt: both 512-chunks + DMA out."""
            tch = tt // 4
            toff = 128 * tt - TCH * tch
            yo = ysb.tile([128, D], F16, tag="y")
            for nch in range(NT):
                ns = slice(TCH * nch, TCH * (nch + 1))
                yps = ps_proj.tile([128, TCH], F32, tag="pj",
                                   name=f"y{tt}_{nch}")
                op_matmuls(yps, tch, toff, ns)
                # the final tiles' copies go on the fast engines (ACT/DVE)
                # and issue their DMA from the ACT queue (skips the busy
                # SP queue at the very end of the kernel)
                eng = nch if tt >= 6 else (tt + nch) % 3
                yo_copy(yo, ns, yps, eng)
                if split_dma:
                    deng = nc.scalar if eng == 0 else nc.sync
                    deng.dma_start(yp_d[128 * tt:128 * (tt + 1), ns],
                                   yo[:, ns])
            if not split_dma:
                nc.sync.dma_start(yp_d[128 * tt:128 * (tt + 1), :], yo[:])

        qk_rest = [(qt_sb, "wq", 2, 0), (kt_sb, "wk", 2, 0),
                   (qt_sb, "wq", 3, 0), (kt_sb, "wk", 3, 0),
                   (qt_sb, "wq", 2, 1), (kt_sb, "wk", 2, 1),
                   (qt_sb, "wq", 3, 1), (kt_sb, "wk", 3, 1)]

        def qk_f(idx):
            dst, wn, et, tch = qk_rest[idx]
            return lambda: emit_qk(dst, wn, et, tch, ps_proj)

        attn_pair(0, 0, v_prefetch=True,
                  fillers=[qk_f(0), None, qk_f(1), None])
        attn_pair(0, 1, fillers=[qk_f(2), qk_f(3)])
        attn_pair(0, 2, fillers=[qk_f(4), qk_f(5)])
        attn_pair(0, 3, fillers=[qk_f(6), qk_f(7)])
        def op_nch(tt, nch):
            """Half of op_tt as a filler unit; DMA fires on the second half."""
            tch = tt // 4
            toff = 128 * tt - TCH * tch
            ns = slice(TCH * nch, TCH * (nch + 1))
            if nch == 0:
                op_nch.yo[tt] = ysb.tile([128, D], F16, tag="y",
                                         name=f"yon{tt}")
            yo = op_nch.yo[tt]
            yps = ps_proj.tile([128, TCH], F32, tag="pj", name=f"y{tt}_{nch}")
            op_matmuls(yps, tch, toff, ns)
            yo_copy(yo, ns, yps, (tt + nch) % 3)
            if nch == NT - 1:
                nc.sync.dma_start(yp_d[128 * tt:128 * (tt + 1), :], yo[:])
        op_nch.yo = {}

        attn_pair(1, 0, fillers=[lambda: op_nch(0, 0), lambda: op_nch(0, 1)])
        attn_pair(1, 1, fillers=[lambda: op_nch(1, 0), lambda: op_nch(1, 1)])
        attn_pair(1, 2, fillers=[lambda: op_nch(2, 0), lambda: op_nch(2, 1)])
        attn_pair(1, 3, fillers=[lambda: op_nch(3, 0), lambda: op_nch(3, 1)],
                  renorm_tts=2)
        for tt in range(4, 8):
            op_tt(tt, split_dma=True)

    nc.compile()
    _NC_CACHE[key] = nc
    return nc


def _f8pair(a):
    f8 = ml_dtypes.float8_e4m3fn
    hi = a.astype(f8)
    lo = (a - hi.astype(np.float32)).astype(f8)
    return hi, lo


def _pm_pairtiles(a):
    """[Ktot, N] -> partition-major [128, Ktot/256, 2, N]:
    out[p, m, i, :] = a[256m + 128i + p, :]."""
    K = a.shape[0]
    rest = a.shape[1:]
    return np.ascontiguousarray(
        a.reshape(K // 256, 2, 128, *rest).transpose(2, 0, 1, 3))


def _prep_core_inputs(x, Wq, Wk, Wv, Wo, Wspan, bspan, cneg, cmask):
    bf = ml_dtypes.bfloat16
    in_maps = []
    # span net on host (f32): a = 1 + z/R per head
    logits = x.mean(axis=1) @ Wspan.T + bspan
    z = T / (1.0 + np.exp(-logits))
    a_full = (1.0 + z / R).astype(np.float16)
    for c in range(N_CORES):
        b, g = c // 2, c % 2
        hs = slice(E * g, E * (g + 1))
        xt = np.ascontiguousarray(x[b].T).astype(np.float32)
        xh, xl = _f8pair(xt)
        xhp = _pm_pairtiles(xh)
        xlp = _pm_pairtiles(xl)
        m = {
            "cmask": cmask,
            "xh0": np.ascontiguousarray(xhp[:, 0:2]),
            "xh1": np.ascontiguousarray(xhp[:, 2:4]),
            "xl0": np.ascontiguousarray(xlp[:, 0:2]),
            "xl1": np.ascontiguousarray(xlp[:, 2:4]),
            "arow": a_full[b, HC * g:HC * (g + 1)].reshape(1, HC),
            "cneg": cneg,
        }
        m["woT"] = np.ascontiguousarray(
            Wo[:, hs].T.reshape(4, 128, D).transpose(1, 0, 2)).astype(bf)
        for wname, W in (("wq", Wq), ("wk", Wk), ("wv", Wv)):
            wt = np.ascontiguousarray(W[hs, :].T).astype(np.float32) * SW
            wh, wl = _f8pair(wt)
            m[wname + "h"] = _pm_pairtiles(wh)
            m[wname + "l"] = _pm_pairtiles(wl)
        in_maps.append(m)
    return in_maps


def _make_c01():
    """cmask: [:, 0:4] causal bias (0 where s' >= j else -1e9), [:, 4] id."""
    sp = np.arange(128, dtype=np.float32)[:, None]
    jp = np.arange(128, dtype=np.float32)[None, :]
    bias = np.where(sp - jp >= 0, 0.0, -1e9).astype(np.float32)
    ident = np.eye(128, dtype=np.float32)
    stk = np.stack([bias, bias, bias, bias, ident])  # [5, 128, 128]
    return np.ascontiguousarray(stk.transpose(1, 0, 2)).astype(
        ml_dtypes.bfloat16)


def _make_cneg(span_full):
    sp = np.arange(128, dtype=np.float32)[:, None]
    cols = []
    for tch in range(NT):
        for k in range(ST):
            tlo = t_lo(k, tch, span_full)
            m_w = span_width(k, tch, span_full)
            if m_w - tlo <= 0:
                continue
            delta = 128 * k - 512 * tch
            tp = np.arange(tlo, m_w, dtype=np.float32)[None, :]
            d = delta + sp - tp
            ramp = np.where(d < 0, -60000.0, -d / R)
            cols.append(ramp)
    if not cols:
        return np.zeros((128, 1), np.float16)
    return np.concatenate(cols, axis=1).astype(np.float16)


def kernel(x, Wq, Wk, Wv, Wo, bo, Wspan, bspan):
    x = np.asarray(x, np.float32)
    Wq = np.asarray(Wq, np.float32)
    Wk = np.asarray(Wk, np.float32)
    Wv = np.asarray(Wv, np.float32)
    Wo = np.asarray(Wo, np.float32)
    bo = np.asarray(bo, np.float32)
    Wspan = np.asarray(Wspan, np.float32)
    bspan = np.asarray(bspan, np.float32)

    # span-mask restriction is only exact when z in [Z_MIN+6, Z_MAX-6]
    logits = x.mean(axis=1) @ Wspan.T + bspan
    z = T / (1.0 + np.exp(-logits))
    span_full = bool(z.min() < Z_MIN + 6.0 or z.max() > Z_MAX - 6.0)
    nc = build_nc(span_full=span_full)
    in_maps = _prep_core_inputs(x, Wq, Wk, Wv, Wo, Wspan, bspan,
                                _make_cneg(span_full), _make_c01())
    res = run_bass_kernel_spmd(nc, in_maps, core_ids=list(range(N_CORES)))
    y = np.empty((B, T, D), np.float32)
    for b in range(B):
        y[b] = (res.results[2 * b]["yp"].astype(np.float32)
                + res.results[2 * b + 1]["yp"].astype(np.float32) + bo)
    return y


# revision 53
# speedup vs baseline: 1.1516x; 1.0179x over previous
"""AdaptiveSpanAttention Trainium2 kernel (8 NeuronCores).

Sharding: core c -> (batch b = c//2, head-group g = c%2).
Each core computes, for its batch and its 8 heads:
  Q/K/V projections in error-compensated fp8 DoubleRow (x and W split
  into fp8e4 hi+lo on host; the 3 significant cross products run with
  pair-slots packing two 128-k-tiles per pass -> 0.75 cycles/row vs
  bf16), anti-causal (j>=i) attention with adaptive-span mask in bf16,
  renormalization, and a partial bf16 output projection
  y_part = Out_g @ Wo[:, e_slice].T.
Host combines: y[b] = yp[2b] + yp[2b+1] + bo  (yp emitted as f16).

The span net (z = T*sigmoid(mean_t x @ WspanT + bspan)) is computed on
host in f32 and shipped as the per-head ramp offset a = 1 + z/R.

Causal zeroing of diagonal blocks is folded into the score PSUM as a
-1e9 bias added by an identity-weight matmul before the exp, so the
exp -> attnV chain has no extra vector-engine stage.

Projection weights are pre-scaled by SW=128 on host so the fp8 lo
residuals stay in e4m3's normal range; the inverse scales fold into
the exp scale and the renorm multiplier.

DMAs are batched (one per tensor, partition-major host layout) because
the cost of a DMA is dominated by a serialized ~650ns issue slot.
"""
import sys

sys.path.insert(0, "/opt/trn_rl_repo")

from contextlib import ExitStack

import ml_dtypes
import numpy as np

import concourse.bass as bass
import concourse.tile as tile
from concourse import bacc, mybir
from concourse.bass_utils import run_bass_kernel_spmd

BF16 = mybir.dt.bfloat16
F16 = mybir.dt.float16
FP8 = mybir.dt.float8e4
F32 = mybir.dt.float32
DR = mybir.MatmulPerfMode.DoubleRow

B, T, D, H = 4, 1024, 1024, 16
DH = 64          # head dim
R = 256.0
HC = 8           # heads per core
E = 512          # channels per core (HC * DH)
N_CORES = 8
TCH = 512        # t-chunk width (PSUM f32 free-dim limit)
NT = T // TCH    # 2 t-chunks
ST = T // 128    # 8 s-tiles
NM = 4           # contraction pair-tiles (1024 = 4 * 256)

SW = 128.0       # host pre-scale on projection weights
OS = 8.0         # attn-out pre-scale before its fp8 hi/lo split
EXP_SCALE = 1.0 / (8.0 * SW * SW)   # folds 1/sqrt(dh) and Q/K weight scales
Y_SCALE = 1.0 / (SW * OS)           # folds Wo and attn-out scales back out

_NC_CACHE = {}

# span-mask restriction bounds, verified on host per call (span_full
# fallback otherwise). z in [Z_MIN+6, Z_MAX-6] required.
Z_MIN = 490.0
Z_MAX = 545.0
CUT = int(R + Z_MAX)  # distance beyond which attention is exactly 0


def causal_width(st, tch):
    """Valid query-column width of block (s_tile=st, t_chunk=tch)."""
    delta = 128 * st - 512 * tch
    return max(0, min(TCH, delta + 128))


def span_width(st, tch, span_full):
    """Columns [0, m_w) where the span mask can differ from 1 (z >= Z_MIN)."""
    delta = 128 * st - 512 * tch
    w = causal_width(st, tch)
    if span_full:
        return w
    return max(0, min(w, delta + 127 - int(Z_MIN)))


def t_lo(st, tch, span_full):
    """Columns [0, t_lo) of the block are fully masked (dist >= R + z)."""
    if span_full:
        return 0
    delta = 128 * st - 512 * tch
    return max(0, delta - CUT)


def build_nc(span_full=False):
    key = ("nc", span_full)
    if key in _NC_CACHE:
        return _NC_CACHE[key]
    nc = bacc.Bacc("TRN2", target_bir_lowering=False, debug=False, num_devices=1)

    # ---- DRAM parameters (per-core shards, partition-major batched) ----
    # x pair tiles: [128 part][NM][2 slots][T]; slot i of pair tile m holds
    # xT rows [256m+128i, 256m+128(i+1)). Split into two halves (m 0-1, 2-3)
    # so the PE can start before the whole tensor lands.
    xh0_d = nc.declare_dram_parameter("xh0", [128, 2, 2, T], FP8, isOutput=False)
    xh1_d = nc.declare_dram_parameter("xh1", [128, 2, 2, T], FP8, isOutput=False)
    xl0_d = nc.declare_dram_parameter("xl0", [128, 2, 2, T], FP8, isOutput=False)
    xl1_d = nc.declare_dram_parameter("xl1", [128, 2, 2, T], FP8, isOutput=False)
    w_d = {}
    for wname in ("wq", "wk", "wv"):
        for lv in ("h", "l"):
            w_d[wname + lv] = nc.declare_dram_parameter(
                wname + lv, [128, NM, 2, E], FP8, isOutput=False)
    woT_d = nc.declare_dram_parameter("woT", [128, 4, D], BF16, isOutput=False)
    # packed span-ramp tiles (see _make_cneg); widths account for the
    # fully-masked column cut
    widths = [max(0, span_width(st, tc, span_full) - t_lo(st, tc, span_full))
              for tc in range(NT) for st in range(ST)]
    offs = np.concatenate([[0], np.cumsum(widths)]).astype(int)
    SMC = max(1, int(offs[-1]))
    smask_d = nc.declare_dram_parameter("smask", [128, HC, SMC], F16,
                                        isOutput=False)
    # cmask[:, k] for k<4: 0 where s' >= j else -1e9 (causal bias);
    # cmask[:, 4] = identity (weights for the bias matmul)
    cmask_d = nc.declare_dram_parameter("cmask", [128, 5, 128], BF16,
                                        isOutput=False)
    yp_d = nc.declare_dram_parameter("yp", [T, D], F16, isOutput=True)

    with tile.TileContext(nc) as tc, ExitStack() as ctx:
        # ---------------- pools ----------------
        consts = ctx.enter_context(tc.tile_pool(name="consts", bufs=1))
        xp = ctx.enter_context(tc.tile_pool(name="xp", bufs=1))
        wp = ctx.enter_context(tc.tile_pool(name="wp", bufs=1))
        qkp = ctx.enter_context(tc.tile_pool(name="qkp", bufs=1))
        vp = ctx.enter_context(tc.tile_pool(name="vp", bufs=1))
        outp = ctx.enter_context(tc.tile_pool(name="outp", bufs=1))
        scr = ctx.enter_context(tc.tile_pool(name="scr", bufs=3))
        ysb = ctx.enter_context(tc.tile_pool(name="ysb", bufs=3))

        lead_ctx = ExitStack()
        ps_lead = lead_ctx.enter_context(
            tc.tile_pool(name="ps_lead", bufs=7, space="PSUM"))
        ps_warm = lead_ctx.enter_context(
            tc.tile_pool(name="ps_warm", bufs=1, space="PSUM"))

        # ---------------- PE p-state warmup ----------------
        # The PE clock ramps with sustained use and resets on idle gaps.
        # Dummy matmuls on a zeroed tile keep it hot through the DMA lead-in.
        warm = consts.tile([128, TCH], BF16)
        nc.vector.memset(warm[:, 0:128], 0.0)
        nc.gpsimd.memset(warm[:, 128:TCH], 0.0)
        wps_holder = [None]

        def dummy(n=1, ap=TCH):
            if wps_holder[0] is None:
                wps_holder[0] = ps_warm.tile([128, TCH], F32, tag="warm",
                                             name="warmps")
            for _ in range(n):
                nc.tensor.matmul(wps_holder[0][:, 0:ap], warm[:, 0:128],
                                 warm[:, 0:ap], start=True, stop=True)

        dummy(3, ap=128)
        dummy(5)

        # ---------------- batched DMA loads ----------------
        xh_sb = xp.tile([128, 2, 2, 2, T], FP8, name="xh_sb")
        xl_sb = xp.tile([128, 2, 2, 2, T], FP8, name="xl_sb")
        nc.sync.dma_start(xh_sb[:, 0], xh0_d[:, :, :, :])
        wq_h = wp.tile([128, NM, 2, E], FP8, name="wq_h")
        nc.sync.dma_start(wq_h[:], w_d["wqh"][:, :, :, :])
        nc.sync.dma_start(xh_sb[:, 1], xh1_d[:, :, :, :])
        wk_h = wp.tile([128, NM, 2, E], FP8, name="wk_h")
        nc.sync.dma_start(wk_h[:], w_d["wkh"][:, :, :, :])
        nc.sync.dma_start(xl_sb[:, 0], xl0_d[:, :, :, :])
        wq_l = wp.tile([128, NM, 2, E], FP8, name="wq_l")
        nc.sync.dma_start(wq_l[:], w_d["wql"][:, :, :, :])
        nc.sync.dma_start(xl_sb[:, 1], xl1_d[:, :, :, :])
        wk_l = wp.tile([128, NM, 2, E], FP8, name="wk_l")
        nc.sync.dma_start(wk_l[:], w_d["wkl"][:, :, :, :])
        cmask_sb = consts.tile([128, 5, 128], BF16, name="cmask_sb")
        nc.sync.dma_start(cmask_sb[:], cmask_d[:, :, :])
        wv_h = wp.tile([128, NM, 2, E], FP8, name="wv_h")
        nc.sync.dma_start(wv_h[:], w_d["wvh"][:, :, :, :])
        wv_l = wp.tile([128, NM, 2, E], FP8, name="wv_l")
        nc.sync.dma_start(wv_l[:], w_d["wvl"][:, :, :, :])
        # span masks split per head pair so pair (0,0) unblocks early
        smask_sb = consts.tile([128, HC, SMC], F16, tag="smask")
        for jp2 in range(4):
            nc.sync.dma_start(smask_sb[:, 2 * jp2:2 * (jp2 + 1), :],
                              smask_d[:, 2 * jp2:2 * (jp2 + 1), :])
        wo_sb = wp.tile([128, 4, D], BF16, name="wo_sb")
        nc.sync.dma_start(wo_sb[:], woT_d[:, :, :])

        def xm(hi, m):
            t_ = xh_sb if hi else xl_sb
            return t_[:, m // 2, m % 2]

        wsb = {"wqh": wq_h, "wql": wq_l, "wkh": wk_h, "wkl": wk_l,
               "wvh": wv_h, "wvl": wv_l}

        # ---------------- Q/K projections (transposed layout) ----------------
        # QT[e, t] = sum_d WqT'[d, e] * xT[d, t] in compensated fp8.
        # Per pair-tile m the 3 products (hi.hi, lo_w.hi_x, hi_w.lo_x) run as
        # DoubleRow passes; lead groups are emitted m-major so the PE chases
        # the DMA stream.
        qt_sb = [qkp.tile([128, T], BF16, tag="qt", name=f"qt{i}", bufs=4)
                 for i in range(4)]
        kt_sb = [qkp.tile([128, T], BF16, tag="kt", name=f"kt{i}", bufs=4)
                 for i in range(4)]

        def qk_mm(ps, wn, et, tch, m, prod, first=False, last=False):
            """One product matmul: prod 0 = hi.hi, 1 = hi_w.lo_x,
            2 = lo_w.hi_x."""
            es = slice(128 * et, 128 * (et + 1))
            ts = slice(TCH * tch, TCH * (tch + 1))
            w_t = wsb[wn + ("h" if prod < 2 else "l")][:, m]
            x_t = xm(1 if prod != 1 else 0, m)
            nc.tensor.matmul(
                ps[:], w_t[:, :, es], x_t[:, :, ts],
                start=first, stop=last, perf_mode=DR, skip_group_check=True)

        def qk_copy(dst_sb, et, tch, ps, eng="act"):
            ts = slice(TCH * tch, TCH * (tch + 1))
            if eng == "act":
                nc.scalar.copy(dst_sb[et][:, ts], ps[:])
            else:
                nc.vector.tensor_copy(dst_sb[et][:, ts], ps[:])

        # 7 lead groups chase the DMA stream in availability order:
        # all hi.hi products (x_hi + W_hi land first), then hi_w.lo_x
        # (x_lo next), then lo_w.hi_x (W_lo last)
        lead_defs = [
            (qt_sb, "wq", 0, 0), (kt_sb, "wk", 0, 0),
            (qt_sb, "wq", 1, 0), (kt_sb, "wk", 1, 0),
            (qt_sb, "wq", 0, 1), (kt_sb, "wk", 0, 1),
            (qt_sb, "wq", 1, 1),
        ]
        lead_ps = [ps_lead.tile([128, TCH], F32, tag="pj", name=f"pl{i}")
                   for i in range(len(lead_defs))]
        for prod in range(2):
            for m in range(NM):
                for gi, (dst, wn, et, tch) in enumerate(lead_defs):
                    qk_mm(lead_ps[gi], wn, et, tch, m, prod,
                          first=(prod == 0 and m == 0))
                if prod == 0:
                    dummy(2)
        # finish group-by-group so qt0/kt0 unlock the attention start early
        for gi, (dst, wn, et, tch) in enumerate(lead_defs):
            for m in range(NM):
                qk_mm(lead_ps[gi], wn, et, tch, m, 2, last=(m == NM - 1))
            qk_copy(dst, et, tch, lead_ps[gi])

        def emit_qk(dst_sb, wn, et, tch, pool):
            ps = pool.tile([128, TCH], F32, tag="pj", name=f"pj{et}_{tch}")
            for m in range(NM):
                for prod in range(3):
                    qk_mm(ps, wn, et, tch, m, prod,
                          first=(m == 0 and prod == 0),
                          last=(m == NM - 1 and prod == 2))
            qk_copy(dst_sb, et, tch, ps, eng="dve")

        # ---------------- V (natural layout, ones-augmented) ----------------
        v_aug = [None] * ST

        def emit_v(st, pool):
            va = vp.tile([128, HC, 2 * DH], BF16, tag="vaug", bufs=ST,
                         name=f"vaug{st}")
            nc.gpsimd.memset(va[:, :, DH:2 * DH], 1.0)
            ps = pool.tile([128, E], F32, tag="pj", name=f"pjv{st}",
                           padded_shape=[128, TCH])
            ss = slice(128 * st, 128 * (st + 1))
            ops = ([(xm(1, m), wv_h[:, m]) for m in range(NM)]
                   + [(xm(1, m), wv_l[:, m]) for m in range(NM)]
                   + [(xm(0, m), wv_h[:, m]) for m in range(NM)])
            for i, (x_t, w_t) in enumerate(ops):
                nc.tensor.matmul(
                    ps[:], x_t[:, :, ss], w_t[:],
                    start=(i == 0), stop=(i == len(ops) - 1), perf_mode=DR,
                    skip_group_check=True)
            nc.vector.tensor_copy(
                va[:, :, 0:DH], ps[:].rearrange("p (h d) -> p h d", h=HC))
            v_aug[st] = va

        emit_qk(kt_sb, "wk", 1, 1, ps_lead)
        emit_v(0, ps_lead)
        emit_v(1, ps_lead)

        lead_ctx.close()
        ps_proj = ctx.enter_context(tc.tile_pool(name="ps_proj", bufs=2, space="PSUM"))
        attn_ctx = ExitStack()
        ps_sc = attn_ctx.enter_context(
            tc.tile_pool(name="ps_sc", bufs=2, space="PSUM"))
        ps_out = attn_ctx.enter_context(
            tc.tile_pool(name="ps_out", bufs=2, space="PSUM"))

        # ---------------- attention ----------------
        # out_pair[j][tch] holds heads 2j (parts 0:64) and 2j+1 (parts 64:128)
        out_pair = [[outp.tile([128, TCH], BF16, tag="out", bufs=8,
                               name=f"op{j}_{c}") for c in range(NT)]
                    for j in range(4)]

        def attn_pair(tch, j, v_prefetch=False, fillers=(), renorm_tts=1):
            """Attention for head pair (2j, 2j+1); both share et=j."""
            first_st = 4 * tch
            heads = (2 * j, 2 * j + 1)
            pouts = [ps_out.tile([128, TCH], F32, tag="pout",
                                 name=f"pout{h}_{tch}") for h in heads]

            def emit_av(st, tlo, w, p_hp):
                for i, h in enumerate(heads):
                    nc.tensor.matmul(
                        pouts[i][:, tlo:w], v_aug[st][:, h, :],
                        p_hp[:, i, tlo:w],
                        start=(st == first_st), stop=(st == ST - 1),
                        skip_group_check=True)

            pending = None
            fillers = list(fillers)
            for st in range(first_st, ST):
                if v_prefetch:
                    if st == first_st and v_aug[st + 2] is None:
                        emit_v(st + 2, ps_proj)
                    if st + 3 < ST and v_aug[st + 3] is None:
                        emit_v(st + 3, ps_proj)
                if fillers:
                    f = fillers.pop(0)
                    if f is not None:
                        f()
                w = causal_width(st, tch)
                k = st - first_st  # delta = 128*k
                tlo = t_lo(st, tch, span_full)
                m_w = span_width(st, tch, span_full)
                moff = offs[8 * tch + st]
                sc_hp = ps_sc.tile([128, 2, TCH], F32, tag="sc",
                                   name=f"sc{j}_{st}")
                diag = k <= 3
                for i, h in enumerate(heads):
                    hp = (h % 2) * 64
                    nc.tensor.matmul(
                        sc_hp[:, i, tlo:w],
                        kt_sb[j][hp:hp + DH, 128 * st:128 * (st + 1)],
                        qt_sb[j][hp:hp + DH, TCH * tch + tlo:TCH * tch + w],
                        start=True, stop=True, skip_group_check=True)
                p_hp = scr.tile([128, 2, TCH], BF16, tag="p", bufs=10,
                                name=f"p{j}_{st}")
                nc.scalar.activation(
                    p_hp[:, :, tlo:w], sc_hp[:, :, tlo:w],
                    mybir.ActivationFunctionType.Exp, scale=EXP_SCALE)
                if diag:
                    # causal zeroing of the diagonal 128x128 sub-block
                    d0 = 128 * k
                    for i, h in enumerate(heads):
                        ceng = nc.gpsimd
                        ceng.tensor_mul(
                            p_hp[:, i, d0:w], p_hp[:, i, d0:w],
                            cmask_sb[:, k, 0:w - d0])
                if m_w > tlo:
                    for i, h in enumerate(heads):
                        # span mask precomputed on host: one 2x-mode multiply
                        nc.vector.tensor_mul(
                            p_hp[:, i, tlo:m_w], p_hp[:, i, tlo:m_w],
                            smask_sb[:, h, moff:moff + m_w - tlo])
                # software pipeline: attnV for the PREVIOUS block runs now,
                # so it never waits on this block's exp/mask chain
                if pending is not None:
                    emit_av(*pending)
                pending = (st, tlo, w, p_hp)
            emit_av(*pending)
            # rows 0:64 numerator (scaled SW); rows 64:128 denominator W
            nchunk = TCH // renorm_tts
            for rchunk in range(renorm_tts):
                cs = slice(rchunk * nchunk, (rchunk + 1) * nchunk)
                for i, h in enumerate(heads):
                    hp = (h % 2) * 64
                    pout = pouts[i]
                    rw = scr.tile([DH, TCH], F32, tag="rw", bufs=8,
                                  name=f"rw{h}_{rchunk}")
                    with nc.allow_low_precision(reason="denom recip"):
                        nc.vector.reciprocal(rw[:, cs], pout[DH:2 * DH, cs])
                    nc.vector.scalar_tensor_tensor(
                        out_pair[j][tch][hp:hp + DH, cs], pout[0:DH, cs],
                        1.0 / SW, rw[:, cs],
                        op0=mybir.AluOpType.mult, op1=mybir.AluOpType.mult)

        def op_matmuls(yps, tch, toff, ns):
            for j in range(4):
                nc.tensor.matmul(
                    yps[:], out_pair[j][tch][:, toff:toff + 128],
                    wo_sb[:, j, ns],
                    start=(j == 0), stop=(j == 3), skip_group_check=True)

        def yo_copy(yo, ns, yps, eng):
            with nc.allow_low_precision(reason="f16 output"):
                if eng % 2 == 0:
                    nc.scalar.copy(yo[:, ns], yps[:])
                else:
                    nc.vector.tensor_copy(yo[:, ns], yps[:])

        def op_tt(tt, split_dma=False, pool=None):
            """Out-projection for t-tile tt: both 512-chunks + DMA out."""
            pool = pool or ps_proj
            tch = tt // 4
            toff = 128 * tt - TCH * tch
            yo = ysb.tile([128, D], F16, tag="y")
            for nch in range(NT):
                ns = slice(TCH * nch, TCH * (nch + 1))
                yps = pool.tile([128, TCH], F32, tag="pj",
                                name=f"y{tt}_{nch}")
                op_matmuls(yps, tch, toff, ns)
                # the final tiles' copies go on the fast engines (ACT/DVE)
                # and issue their DMA from the ACT queue (skips the busy
                # SP queue at the very end of the kernel)
                eng = nch if tt >= 6 else (tt + nch) % 3
                yo_copy(yo, ns, yps, eng)
                if split_dma:
                    deng = nc.scalar if eng == 0 else nc.sync
                    deng.dma_start(yp_d[128 * tt:128 * (tt + 1), ns],
                                   yo[:, ns])
            if not split_dma:
                nc.sync.dma_start(yp_d[128 * tt:128 * (tt + 1), :], yo[:])

        qk_rest = [(qt_sb, "wq", 2, 0), (kt_sb, "wk", 2, 0),
                   (qt_sb, "wq", 3, 0), (kt_sb, "wk", 3, 0),
                   (qt_sb, "wq", 2, 1), (kt_sb, "wk", 2, 1),
                   (qt_sb, "wq", 3, 1), (kt_sb, "wk", 3, 1)]

        def qk_f(idx):
            dst, wn, et, tch = qk_rest[idx]
            return lambda: emit_qk(dst, wn, et, tch, ps_proj)

        attn_pair(0, 0, v_prefetch=True,
                  fillers=[qk_f(0), None, qk_f(1), None])
        attn_pair(0, 1, fillers=[qk_f(2), qk_f(3)])
        attn_pair(0, 2, fillers=[qk_f(4), qk_f(5)])
        attn_pair(0, 3, fillers=[qk_f(6), qk_f(7)])
        def op_nch(tt, nch):
            """Half of op_tt as a filler unit; DMA fires on the second half."""
            tch = tt // 4
            toff = 128 * tt - TCH * tch
            ns = slice(TCH * nch, TCH * (nch + 1))
            if nch == 0:
                op_nch.yo[tt] = ysb.tile([128, D], F16, tag="y",
                                         name=f"yon{tt}")
            yo = op_nch.yo[tt]
            yps = ps_proj.tile([128, TCH], F32, tag="pj", name=f"y{tt}_{nch}")
            op_matmuls(yps, tch, toff, ns)
            yo_copy(yo, ns, yps, (tt + nch) % 3)
            if nch == NT - 1:
                nc.sync.dma_start(yp_d[128 * tt:128 * (tt + 1), :], yo[:])
        op_nch.yo = {}

        attn_pair(1, 0, fillers=[lambda: op_nch(0, 0), lambda: op_nch(0, 1)])
        attn_pair(1, 1, fillers=[lambda: op_nch(1, 0), lambda: op_nch(1, 1)])
        attn_pair(1, 2, fillers=[lambda: op_nch(2, 0), lambda: op_nch(2, 1)])
        attn_pair(1, 3, fillers=[lambda: op_nch(3, 0)], renorm_tts=4)
        op_nch(3, 1)
        # attention PSUM pools are done; hand their banks to the tail
        # out-projections so four groups can be in flight
        attn_ctx.close()
        ps_tail = ctx.enter_context(
            tc.tile_pool(name="ps_tail", bufs=4, space="PSUM"))
        for tt in range(4, 8):
            op_tt(tt, split_dma=True, pool=ps_tail)

    nc.compile()
    _NC_CACHE[key] = nc
    return nc


def _f8pair(a):
    f8 = ml_dtypes.float8_e4m3fn
    hi = a.astype(f8)
    lo = (a - hi.astype(np.float32)).astype(f8)
    return hi, lo


def _pm_pairtiles(a):
    """[Ktot, N] -> partition-major [128, Ktot/256, 2, N]:
    out[p, m, i, :] = a[256m + 128i + p, :]."""
    K = a.shape[0]
    rest = a.shape[1:]
    return np.ascontiguousarray(
        a.reshape(K // 256, 2, 128, *rest).transpose(2, 0, 1, 3))


def _prep_core_inputs(x, Wq, Wk, Wv, Wo, Wspan, bspan, span_full, cmask):
    bf = ml_dtypes.bfloat16
    in_maps = []
    # span net on host (f32): z per (batch, head)
    logits = x.mean(axis=1) @ Wspan.T + bspan
    z = T / (1.0 + np.exp(-logits))
    for c in range(N_CORES):
        b, g = c // 2, c % 2
        hs = slice(E * g, E * (g + 1))
        xt = np.ascontiguousarray(x[b].T).astype(np.float32)
        xh, xl = _f8pair(xt)
        xhp = _pm_pairtiles(xh)
        xlp = _pm_pairtiles(xl)
        m = {
            "cmask": cmask,
            "xh0": np.ascontiguousarray(xhp[:, 0:2]),
            "xh1": np.ascontiguousarray(xhp[:, 2:4]),
            "xl0": np.ascontiguousarray(xlp[:, 0:2]),
            "xl1": np.ascontiguousarray(xlp[:, 2:4]),
            "smask": _make_smask(z[b, HC * g:HC * (g + 1)], span_full),
        }
        m["woT"] = np.ascontiguousarray(
            Wo[:, hs].T.reshape(4, 128, D).transpose(1, 0, 2)).astype(bf)
        for wname, W in (("wq", Wq), ("wk", Wk), ("wv", Wv)):
            wt = np.ascontiguousarray(W[hs, :].T).astype(np.float32) * SW
            wh, wl = _f8pair(wt)
            m[wname + "h"] = _pm_pairtiles(wh)
            m[wname + "l"] = _pm_pairtiles(wl)
        in_maps.append(m)
    return in_maps


def _make_c01():
    """cmask: [:, 0:4] causal 0/1 (1 where s' >= j), [:, 4] identity."""
    sp = np.arange(128, dtype=np.float32)[:, None]
    jp = np.arange(128, dtype=np.float32)[None, :]
    c01 = (sp - jp >= 0).astype(np.float32)
    ident = np.eye(128, dtype=np.float32)
    stk = np.stack([c01, c01, c01, c01, ident])  # [5, 128, 128]
    return np.ascontiguousarray(stk.transpose(1, 0, 2)).astype(
        ml_dtypes.bfloat16)


def _make_smask(z_heads, span_full):
    """Per-head span masks clip((R + z - d)/R, 0, 1), packed like the
    kernel's block windows: [128 s', HC, total span cols]."""
    sp = np.arange(128, dtype=np.float32)[:, None]
    cols = []
    for tch in range(NT):
        for k in range(ST):
            tlo = t_lo(k, tch, span_full)
            m_w = span_width(k, tch, span_full)
            if m_w - tlo <= 0:
                continue
            delta = 128 * k - 512 * tch
            tp = np.arange(tlo, m_w, dtype=np.float32)[None, :]
            d = delta + sp - tp
            cols.append(np.where(d < 0, 0.0, d))
    if not cols:
        return np.zeros((128, HC, 1), np.float16)
    dall = np.concatenate(cols, axis=1)  # [128, S]
    mask = np.clip((R + z_heads[None, :, None] - dall[:, None, :]) / R,
                   0.0, 1.0)
    return mask.astype(np.float16)


def kernel(x, Wq, Wk, Wv, Wo, bo, Wspan, bspan):
    x = np.asarray(x, np.float32)
    Wq = np.asarray(Wq, np.float32)
    Wk = np.asarray(Wk, np.float32)
    Wv = np.asarray(Wv, np.float32)
    Wo = np.asarray(Wo, np.float32)
    bo = np.asarray(bo, np.float32)
    Wspan = np.asarray(Wspan, np.float32)
    bspan = np.asarray(bspan, np.float32)

    # span-mask restriction is only exact when z in [Z_MIN+6, Z_MAX-6]
    logits = x.mean(axis=1) @ Wspan.T + bspan
    z = T / (1.0 + np.exp(-logits))
    span_full = bool(z.min() < Z_MIN + 6.0 or z.max() > Z_MAX - 6.0)
    nc = build_nc(span_full=span_full)
    in_maps = _prep_core_inputs(x, Wq, Wk, Wv, Wo, Wspan, bspan,
                                span_full, _make_c01())
    res = run_bass_kernel_spmd(nc, in_maps, core_ids=list(range(N_CORES)))
    y = np.empty((B, T, D), np.float32)
    for b in range(B):
        y[b] = (res.results[2 * b]["yp"].astype(np.float32)
                + res.results[2 * b + 1]["yp"].astype(np.float32) + bo)
    return y


# revision 57
# speedup vs baseline: 1.1853x; 1.0292x over previous
"""AdaptiveSpanAttention Trainium2 kernel (8 NeuronCores).

Sharding: core c -> (batch b = c//2, head-group g = c%2).
Each core computes, for its batch and its 8 heads:
  Q/K/V projections in error-compensated fp8 DoubleRow (x and W split
  into fp8e4 hi+lo on host; the 3 significant cross products run with
  pair-slots packing two 128-k-tiles per pass -> 0.75 cycles/row vs
  bf16), anti-causal (j>=i) attention with adaptive-span mask in bf16,
  renormalization, and a partial bf16 output projection
  y_part = Out_g @ Wo[:, e_slice].T.
Host combines: y[b] = yp[2b] + yp[2b+1] + bo  (yp emitted as f16).

The span net (z = T*sigmoid(mean_t x @ WspanT + bspan)) is computed on
host in f32 and shipped as the per-head ramp offset a = 1 + z/R.

Causal zeroing of diagonal blocks is folded into the score PSUM as a
-1e9 bias added by an identity-weight matmul before the exp, so the
exp -> attnV chain has no extra vector-engine stage.

Projection weights are pre-scaled by SW=128 on host so the fp8 lo
residuals stay in e4m3's normal range; the inverse scales fold into
the exp scale and the renorm multiplier.

DMAs are batched (one per tensor, partition-major host layout) because
the cost of a DMA is dominated by a serialized ~650ns issue slot.
"""
import sys

sys.path.insert(0, "/opt/trn_rl_repo")

from contextlib import ExitStack

import ml_dtypes
import numpy as np

import concourse.bass as bass
import concourse.tile as tile
from concourse import bacc, mybir
from concourse.bass_utils import run_bass_kernel_spmd

BF16 = mybir.dt.bfloat16
F16 = mybir.dt.float16
FP8 = mybir.dt.float8e4
F32 = mybir.dt.float32
DR = mybir.MatmulPerfMode.DoubleRow

B, T, D, H = 4, 1024, 1024, 16
DH = 64          # head dim
R = 256.0
HC = 8           # heads per core
E = 512          # channels per core (HC * DH)
N_CORES = 8
TCH = 512        # t-chunk width (PSUM f32 free-dim limit)
NT = T // TCH    # 2 t-chunks
ST = T // 128    # 8 s-tiles
NM = 4           # contraction pair-tiles (1024 = 4 * 256)

SW = 128.0       # host pre-scale on projection weights
OS = 8.0         # attn-out pre-scale before its fp8 hi/lo split
EXP_SCALE = 1.0 / (8.0 * SW * SW)   # folds 1/sqrt(dh) and Q/K weight scales
Y_SCALE = 1.0 / (SW * OS)           # folds Wo and attn-out scales back out

_NC_CACHE = {}

# span-mask restriction bounds, verified on host per call (span_full
# fallback otherwise). z in [Z_MIN+6, Z_MAX-6] required.
Z_MIN = 490.0
Z_MAX = 545.0
CUT = int(R + Z_MAX)  # distance beyond which attention is exactly 0


def causal_width(st, tch):
    """Valid query-column width of block (s_tile=st, t_chunk=tch)."""
    delta = 128 * st - 512 * tch
    return max(0, min(TCH, delta + 128))


def span_width(st, tch, span_full):
    """Columns [0, m_w) where the span mask can differ from 1 (z >= Z_MIN)."""
    delta = 128 * st - 512 * tch
    w = causal_width(st, tch)
    if span_full:
        return w
    return max(0, min(w, delta + 127 - int(Z_MIN)))


def t_lo(st, tch, span_full):
    """Columns [0, t_lo) of the block are fully masked (dist >= R + z)."""
    if span_full:
        return 0
    delta = 128 * st - 512 * tch
    return max(0, delta - CUT)


def build_nc(span_full=False):
    key = ("nc", span_full)
    if key in _NC_CACHE:
        return _NC_CACHE[key]
    nc = bacc.Bacc("TRN2", target_bir_lowering=False, debug=False, num_devices=1)

    # ---- DRAM parameters (per-core shards, partition-major batched) ----
    # x pair tiles: [128 part][NM][2 slots][T]; slot i of pair tile m holds
    # xT rows [256m+128i, 256m+128(i+1)). Split into two halves (m 0-1, 2-3)
    # so the PE can start before the whole tensor lands.
    xh0_d = nc.declare_dram_parameter("xh0", [128, 2, 2, T], FP8, isOutput=False)
    xh1_d = nc.declare_dram_parameter("xh1", [128, 2, 2, T], FP8, isOutput=False)
    xl0_d = nc.declare_dram_parameter("xl0", [128, 2, 2, T], FP8, isOutput=False)
    xl1_d = nc.declare_dram_parameter("xl1", [128, 2, 2, T], FP8, isOutput=False)
    w_d = {}
    for wname in ("wq", "wk", "wv"):
        for lv in ("h", "l"):
            w_d[wname + lv] = nc.declare_dram_parameter(
                wname + lv, [128, NM, 2, E], FP8, isOutput=False)
    woT_d = nc.declare_dram_parameter("woT", [128, 4, D], BF16, isOutput=False)
    # packed span-ramp tiles (see _make_cneg); widths account for the
    # fully-masked column cut
    widths = [max(0, span_width(st, tc, span_full) - t_lo(st, tc, span_full))
              for tc in range(NT) for st in range(ST)]
    offs = np.concatenate([[0], np.cumsum(widths)]).astype(int)
    SMC = max(1, int(offs[-1]))
    smask_d = nc.declare_dram_parameter("smask", [128, HC, SMC], F16,
                                        isOutput=False)
    # cmask[:, k] for k<4: 0 where s' >= j else -1e9 (causal bias);
    # cmask[:, 4] = identity (weights for the bias matmul)
    cmask_d = nc.declare_dram_parameter("cmask", [128, 5, 128], BF16,
                                        isOutput=False)
    yp_d = nc.declare_dram_parameter("yp", [T, D], F16, isOutput=True)

    with tile.TileContext(nc) as tc, ExitStack() as ctx:
        # ---------------- pools ----------------
        consts = ctx.enter_context(tc.tile_pool(name="consts", bufs=1))
        xp = ctx.enter_context(tc.tile_pool(name="xp", bufs=1))
        wp = ctx.enter_context(tc.tile_pool(name="wp", bufs=1))
        qkp = ctx.enter_context(tc.tile_pool(name="qkp", bufs=1))
        vp = ctx.enter_context(tc.tile_pool(name="vp", bufs=1))
        outp = ctx.enter_context(tc.tile_pool(name="outp", bufs=1))
        scr = ctx.enter_context(tc.tile_pool(name="scr", bufs=3))
        ysb = ctx.enter_context(tc.tile_pool(name="ysb", bufs=4))

        lead_ctx = ExitStack()
        ps_lead = lead_ctx.enter_context(
            tc.tile_pool(name="ps_lead", bufs=7, space="PSUM"))
        ps_warm = lead_ctx.enter_context(
            tc.tile_pool(name="ps_warm", bufs=1, space="PSUM"))

        # ---------------- PE p-state warmup ----------------
        # The PE clock ramps with sustained use and resets on idle gaps.
        # Dummy matmuls on a zeroed tile keep it hot through the DMA lead-in.
        warm = consts.tile([128, TCH], BF16)
        nc.vector.memset(warm[:, 0:128], 0.0)
        nc.gpsimd.memset(warm[:, 128:TCH], 0.0)
        wps_holder = [None]

        def dummy(n=1, ap=TCH):
            if wps_holder[0] is None:
                wps_holder[0] = ps_warm.tile([128, TCH], F32, tag="warm",
                                             name="warmps")
            for _ in range(n):
                nc.tensor.matmul(wps_holder[0][:, 0:ap], warm[:, 0:128],
                                 warm[:, 0:ap], start=True, stop=True)

        dummy(3, ap=128)
        dummy(5)

        # ---------------- batched DMA loads ----------------
        xh_sb = xp.tile([128, 2, 2, 2, T], FP8, name="xh_sb")
        xl_sb = xp.tile([128, 2, 2, 2, T], FP8, name="xl_sb")
        nc.sync.dma_start(xh_sb[:, 0], xh0_d[:, :, :, :])
        wq_h = wp.tile([128, NM, 2, E], FP8, name="wq_h")
        nc.sync.dma_start(wq_h[:], w_d["wqh"][:, :, :, :])
        wk_h = wp.tile([128, NM, 2, E], FP8, name="wk_h")
        nc.sync.dma_start(wk_h[:], w_d["wkh"][:, :, :, :])
        nc.sync.dma_start(xh_sb[:, 1], xh1_d[:, :, :, :])
        nc.sync.dma_start(xl_sb[:, 0], xl0_d[:, :, :, :])
        nc.sync.dma_start(xl_sb[:, 1], xl1_d[:, :, :, :])
        wq_l = wp.tile([128, NM, 2, E], FP8, name="wq_l")
        nc.sync.dma_start(wq_l[:], w_d["wql"][:, :, :, :])
        wk_l = wp.tile([128, NM, 2, E], FP8, name="wk_l")
        nc.sync.dma_start(wk_l[:], w_d["wkl"][:, :, :, :])
        cmask_sb = consts.tile([128, 5, 128], BF16, name="cmask_sb")
        nc.sync.dma_start(cmask_sb[:], cmask_d[:, :, :])
        wv_h = wp.tile([128, NM, 2, E], FP8, name="wv_h")
        nc.sync.dma_start(wv_h[:], w_d["wvh"][:, :, :, :])
        wv_l = wp.tile([128, NM, 2, E], FP8, name="wv_l")
        nc.sync.dma_start(wv_l[:], w_d["wvl"][:, :, :, :])
        # span masks split per head pair so pair (0,0) unblocks early
        smask_sb = consts.tile([128, HC, SMC], F16, tag="smask")
        for jp2 in range(4):
            nc.sync.dma_start(smask_sb[:, 2 * jp2:2 * (jp2 + 1), :],
                              smask_d[:, 2 * jp2:2 * (jp2 + 1), :])
        wo_sb = wp.tile([128, 4, D], BF16, name="wo_sb")
        nc.sync.dma_start(wo_sb[:], woT_d[:, :, :])

        def xm(hi, m):
            t_ = xh_sb if hi else xl_sb
            return t_[:, m // 2, m % 2]

        wsb = {"wqh": wq_h, "wql": wq_l, "wkh": wk_h, "wkl": wk_l,
               "wvh": wv_h, "wvl": wv_l}

        # ---------------- Q/K projections (transposed layout) ----------------
        # QT[e, t] = sum_d WqT'[d, e] * xT[d, t] in compensated fp8.
        # Per pair-tile m the 3 products (hi.hi, lo_w.hi_x, hi_w.lo_x) run as
        # DoubleRow passes; lead groups are emitted m-major so the PE chases
        # the DMA stream.
        qt_sb = [qkp.tile([128, T], BF16, tag="qt", name=f"qt{i}", bufs=4)
                 for i in range(4)]
        kt_sb = [qkp.tile([128, T], BF16, tag="kt", name=f"kt{i}", bufs=4)
                 for i in range(4)]

        def qk_mm(ps, wn, et, tch, m, prod, first=False, last=False):
            """One product matmul: prod 0 = hi.hi, 1 = hi_w.lo_x,
            2 = lo_w.hi_x."""
            es = slice(128 * et, 128 * (et + 1))
            ts = slice(TCH * tch, TCH * (tch + 1))
            w_t = wsb[wn + ("h" if prod < 2 else "l")][:, m]
            x_t = xm(1 if prod != 1 else 0, m)
            nc.tensor.matmul(
                ps[:], w_t[:, :, es], x_t[:, :, ts],
                start=first, stop=last, perf_mode=DR, skip_group_check=True)

        def qk_copy(dst_sb, et, tch, ps, eng="act"):
            ts = slice(TCH * tch, TCH * (tch + 1))
            if eng == "act":
                nc.scalar.copy(dst_sb[et][:, ts], ps[:])
            else:
                nc.vector.tensor_copy(dst_sb[et][:, ts], ps[:])

        # 7 lead groups chase the DMA stream in availability order:
        # all hi.hi products (x_hi + W_hi land first), then hi_w.lo_x
        # (x_lo next), then lo_w.hi_x (W_lo last)
        lead_defs = [
            (qt_sb, "wq", 0, 0), (kt_sb, "wk", 0, 0),
            (qt_sb, "wq", 1, 0), (kt_sb, "wk", 1, 0),
            (qt_sb, "wq", 0, 1), (kt_sb, "wk", 0, 1),
            (qt_sb, "wq", 1, 1),
        ]
        lead_ps = [ps_lead.tile([128, TCH], F32, tag="pj", name=f"pl{i}")
                   for i in range(len(lead_defs))]
        for prod in range(2):
            for m in range(NM):
                for gi, (dst, wn, et, tch) in enumerate(lead_defs):
                    qk_mm(lead_ps[gi], wn, et, tch, m, prod,
                          first=(prod == 0 and m == 0))
                if prod == 0:
                    dummy(3 if m >= 2 else 2)
        # finish group-by-group so qt0/kt0 unlock the attention start early
        for gi, (dst, wn, et, tch) in enumerate(lead_defs):
            for m in range(NM):
                qk_mm(lead_ps[gi], wn, et, tch, m, 2, last=(m == NM - 1))
            qk_copy(dst, et, tch, lead_ps[gi])

        def emit_qk(dst_sb, wn, et, tch, pool):
            ps = pool.tile([128, TCH], F32, tag="pj", name=f"pj{et}_{tch}")
            for m in range(NM):
                for prod in range(3):
                    qk_mm(ps, wn, et, tch, m, prod,
                          first=(m == 0 and prod == 0),
                          last=(m == NM - 1 and prod == 2))
            qk_copy(dst_sb, et, tch, ps, eng="dve")

        # ---------------- V (natural layout, ones-augmented) ----------------
        v_aug = [None] * ST

        def emit_v(st, pool):
            va = vp.tile([128, HC, 2 * DH], BF16, tag="vaug", bufs=ST,
                         name=f"vaug{st}")
            nc.gpsimd.memset(va[:, :, DH:2 * DH], 1.0)
            ps = pool.tile([128, E], F32, tag="pj", name=f"pjv{st}",
                           padded_shape=[128, TCH])
            ss = slice(128 * st, 128 * (st + 1))
            ops = ([(xm(1, m), wv_h[:, m]) for m in range(NM)]
                   + [(xm(1, m), wv_l[:, m]) for m in range(NM)]
                   + [(xm(0, m), wv_h[:, m]) for m in range(NM)])
            for i, (x_t, w_t) in enumerate(ops):
                nc.tensor.matmul(
                    ps[:], x_t[:, :, ss], w_t[:],
                    start=(i == 0), stop=(i == len(ops) - 1), perf_mode=DR,
                    skip_group_check=True)
            nc.vector.tensor_copy(
                va[:, :, 0:DH], ps[:].rearrange("p (h d) -> p h d", h=HC))
            v_aug[st] = va

        emit_qk(kt_sb, "wk", 1, 1, ps_lead)
        emit_v(0, ps_lead)
        emit_v(1, ps_lead)

        lead_ctx.close()
        ps_proj = ctx.enter_context(tc.tile_pool(name="ps_proj", bufs=2, space="PSUM"))
        attn_ctx = ExitStack()
        ps_sc = attn_ctx.enter_context(
            tc.tile_pool(name="ps_sc", bufs=2, space="PSUM"))
        ps_out = attn_ctx.enter_context(
            tc.tile_pool(name="ps_out", bufs=2, space="PSUM"))

        # ---------------- attention ----------------
        # out_pair[j][tch] holds heads 2j (parts 0:64) and 2j+1 (parts 64:128)
        out_pair = [[outp.tile([128, TCH], BF16, tag="out", bufs=8,
                               name=f"op{j}_{c}") for c in range(NT)]
                    for j in range(4)]

        def attn_pair(tch, j, v_prefetch=False, fillers=(), renorm_tts=1):
            """Attention for head pair (2j, 2j+1); both share et=j."""
            first_st = 4 * tch
            heads = (2 * j, 2 * j + 1)
            pouts = [ps_out.tile([128, TCH], F32, tag="pout",
                                 name=f"pout{h}_{tch}") for h in heads]

            def emit_av(st, tlo, w, p_hp):
                for i, h in enumerate(heads):
                    nc.tensor.matmul(
                        pouts[i][:, tlo:w], v_aug[st][:, h, :],
                        p_hp[:, i, tlo:w],
                        start=(st == first_st), stop=(st == ST - 1),
                        skip_group_check=True)

            pending = None
            fillers = list(fillers)
            for st in range(first_st, ST):
                if v_prefetch:
                    if st == first_st and v_aug[st + 2] is None:
                        emit_v(st + 2, ps_proj)
                    if st + 3 < ST and v_aug[st + 3] is None:
                        emit_v(st + 3, ps_proj)
                if fillers:
                    f = fillers.pop(0)
                    if f is not None:
                        f()
                w = causal_width(st, tch)
                k = st - first_st  # delta = 128*k
                tlo = t_lo(st, tch, span_full)
                m_w = span_width(st, tch, span_full)
                moff = offs[8 * tch + st]
                sc_hp = ps_sc.tile([128, 2, TCH], F32, tag="sc",
                                   name=f"sc{j}_{st}")
                diag = k <= 3
                for i, h in enumerate(heads):
                    hp = (h % 2) * 64
                    nc.tensor.matmul(
                        sc_hp[:, i, tlo:w],
                        kt_sb[j][hp:hp + DH, 128 * st:128 * (st + 1)],
                        qt_sb[j][hp:hp + DH, TCH * tch + tlo:TCH * tch + w],
                        start=True, stop=True, skip_group_check=True)
                p_hp = scr.tile([128, 2, TCH], BF16, tag="p", bufs=10,
                                name=f"p{j}_{st}")
                nc.scalar.activation(
                    p_hp[:, :, tlo:w], sc_hp[:, :, tlo:w],
                    mybir.ActivationFunctionType.Exp, scale=EXP_SCALE)
                if diag:
                    # causal zeroing of the diagonal 128x128 sub-block
                    d0 = 128 * k
                    for i, h in enumerate(heads):
                        ceng = nc.gpsimd
                        ceng.tensor_mul(
                            p_hp[:, i, d0:w], p_hp[:, i, d0:w],
                            cmask_sb[:, k, 0:w - d0])
                if m_w > tlo:
                    for i, h in enumerate(heads):
                        # span mask precomputed on host: one 2x-mode multiply
                        nc.vector.tensor_mul(
                            p_hp[:, i, tlo:m_w], p_hp[:, i, tlo:m_w],
                            smask_sb[:, h, moff:moff + m_w - tlo])
                # software pipeline: attnV for the PREVIOUS block runs now,
                # so it never waits on this block's exp/mask chain
                if pending is not None:
                    emit_av(*pending)
                pending = (st, tlo, w, p_hp)
            emit_av(*pending)
            # rows 0:64 numerator (scaled SW); rows 64:128 denominator W
            nchunk = TCH // renorm_tts
            for rchunk in range(renorm_tts):
                cs = slice(rchunk * nchunk, (rchunk + 1) * nchunk)
                for i, h in enumerate(heads):
                    hp = (h % 2) * 64
                    pout = pouts[i]
                    rw = scr.tile([DH, TCH], F32, tag="rw", bufs=4,
                                  name=f"rw{h}_{rchunk}")
                    with nc.allow_low_precision(reason="denom recip"):
                        nc.vector.reciprocal(rw[:, cs], pout[DH:2 * DH, cs])
                    nc.vector.scalar_tensor_tensor(
                        out_pair[j][tch][hp:hp + DH, cs], pout[0:DH, cs],
                        1.0 / SW, rw[:, cs],
                        op0=mybir.AluOpType.mult, op1=mybir.AluOpType.mult)

        def op_matmuls(yps, tch, toff, ns):
            for j in range(4):
                nc.tensor.matmul(
                    yps[:], out_pair[j][tch][:, toff:toff + 128],
                    wo_sb[:, j, ns],
                    start=(j == 0), stop=(j == 3), skip_group_check=True)

        def yo_copy(yo, ns, yps, eng):
            with nc.allow_low_precision(reason="f16 output"):
                if eng % 2 == 0:
                    nc.scalar.copy(yo[:, ns], yps[:])
                else:
                    nc.vector.tensor_copy(yo[:, ns], yps[:])

        def op_tt(tt, split_dma=False, pool=None):
            """Out-projection for t-tile tt: both 512-chunks + DMA out."""
            pool = pool or ps_proj
            tch = tt // 4
            toff = 128 * tt - TCH * tch
            yo = ysb.tile([128, D], F16, tag="y")
            for nch in range(NT):
                ns = slice(TCH * nch, TCH * (nch + 1))
                yps = pool.tile([128, TCH], F32, tag="pj",
                                name=f"y{tt}_{nch}")
                op_matmuls(yps, tch, toff, ns)
                # the final tiles' copies go on the fast engines (ACT/DVE)
                # and issue their DMA from the ACT queue (skips the busy
                # SP queue at the very end of the kernel)
                eng = nch if tt >= 6 else (tt + nch) % 3
                yo_copy(yo, ns, yps, eng)
                if split_dma:
                    deng = nc.scalar if eng == 0 else nc.sync
                    deng.dma_start(yp_d[128 * tt:128 * (tt + 1), ns],
                                   yo[:, ns])
            if not split_dma:
                nc.sync.dma_start(yp_d[128 * tt:128 * (tt + 1), :], yo[:])

        qk_rest = [(qt_sb, "wq", 2, 0), (kt_sb, "wk", 2, 0),
                   (qt_sb, "wq", 3, 0), (kt_sb, "wk", 3, 0),
                   (qt_sb, "wq", 2, 1), (kt_sb, "wk", 2, 1),
                   (qt_sb, "wq", 3, 1), (kt_sb, "wk", 3, 1)]

        def qk_f(idx):
            dst, wn, et, tch = qk_rest[idx]
            return lambda: emit_qk(dst, wn, et, tch, ps_proj)

        attn_pair(0, 0, v_prefetch=True,
                  fillers=[qk_f(0), None, qk_f(1), None])
        attn_pair(0, 1, fillers=[qk_f(2), qk_f(3)])
        attn_pair(0, 2, fillers=[qk_f(4), qk_f(5)])
        attn_pair(0, 3, fillers=[qk_f(6), qk_f(7)])
        def op_nch(tt, nch):
            """Half of op_tt as a filler unit; DMA fires on the second half."""
            tch = tt // 4
            toff = 128 * tt - TCH * tch
            ns = slice(TCH * nch, TCH * (nch + 1))
            if nch == 0:
                op_nch.yo[tt] = ysb.tile([128, D], F16, tag="y",
                                         name=f"yon{tt}")
            yo = op_nch.yo[tt]
            yps = ps_proj.tile([128, TCH], F32, tag="pj", name=f"y{tt}_{nch}")
            op_matmuls(yps, tch, toff, ns)
            yo_copy(yo, ns, yps, (tt + nch) % 3)
            if nch == NT - 1:
                nc.sync.dma_start(yp_d[128 * tt:128 * (tt + 1), :], yo[:])
        op_nch.yo = {}

        attn_pair(1, 0, fillers=[lambda: op_nch(0, 0), lambda: op_nch(0, 1)])
        attn_pair(1, 1, fillers=[lambda: op_nch(1, 0), lambda: op_nch(1, 1)])
        attn_pair(1, 2, fillers=[lambda: op_nch(2, 0), lambda: op_nch(2, 1)])
        attn_pair(1, 3, fillers=[lambda: op_nch(3, 0), None,
                                 lambda: dummy(3, ap=256)], renorm_tts=4)
        op_nch(3, 1)
        # attention PSUM pools are done; hand their banks to the tail
        # out-projections so four groups can be in flight
        attn_ctx.close()
        ps_tail = ctx.enter_context(
            tc.tile_pool(name="ps_tail", bufs=4, space="PSUM"))
        for tt in range(4, 8):
            op_tt(tt, pool=ps_tail)

    nc.compile()
    _NC_CACHE[key] = nc
    return nc


def _f8pair(a):
    f8 = ml_dtypes.float8_e4m3fn
    hi = a.astype(f8)
    lo = (a - hi.astype(np.float32)).astype(f8)
    return hi, lo


def _pm_pairtiles(a):
    """[Ktot, N] -> partition-major [128, Ktot/256, 2, N]:
    out[p, m, i, :] = a[256m + 128i + p, :]."""
    K = a.shape[0]
    rest = a.shape[1:]
    return np.ascontiguousarray(
        a.reshape(K // 256, 2, 128, *rest).transpose(2, 0, 1, 3))


def _prep_core_inputs(x, Wq, Wk, Wv, Wo, Wspan, bspan, span_full, cmask):
    bf = ml_dtypes.bfloat16
    in_maps = []
    # span net on host (f32): z per (batch, head)
    logits = x.mean(axis=1) @ Wspan.T + bspan
    z = T / (1.0 + np.exp(-logits))
    for c in range(N_CORES):
        b, g = c // 2, c % 2
        hs = slice(E * g, E * (g + 1))
        xt = np.ascontiguousarray(x[b].T).astype(np.float32)
        xh, xl = _f8pair(xt)
        xhp = _pm_pairtiles(xh)
        xlp = _pm_pairtiles(xl)
        m = {
            "cmask": cmask,
            "xh0": np.ascontiguousarray(xhp[:, 0:2]),
            "xh1": np.ascontiguousarray(xhp[:, 2:4]),
            "xl0": np.ascontiguousarray(xlp[:, 0:2]),
            "xl1": np.ascontiguousarray(xlp[:, 2:4]),
            "smask": _make_smask(z[b, HC * g:HC * (g + 1)], span_full),
        }
        m["woT"] = np.ascontiguousarray(
            Wo[:, hs].T.reshape(4, 128, D).transpose(1, 0, 2)).astype(bf)
        for wname, W in (("wq", Wq), ("wk", Wk), ("wv", Wv)):
            wt = np.ascontiguousarray(W[hs, :].T).astype(np.float32) * SW
            wh, wl = _f8pair(wt)
            m[wname + "h"] = _pm_pairtiles(wh)
            m[wname + "l"] = _pm_pairtiles(wl)
        in_maps.append(m)
    return in_maps


def _make_c01():
    """cmask: [:, 0:4] causal 0/1 (1 where s' >= j), [:, 4] identity."""
    sp = np.arange(128, dtype=np.float32)[:, None]
    jp = np.arange(128, dtype=np.float32)[None, :]
    c01 = (sp - jp >= 0).astype(np.float32)
    ident = np.eye(128, dtype=np.float32)
    stk = np.stack([c01, c01, c01, c01, ident])  # [5, 128, 128]
    return np.ascontiguousarray(stk.transpose(1, 0, 2)).astype(
        ml_dtypes.bfloat16)


def _make_smask(z_heads, span_full):
    """Per-head span masks clip((R + z - d)/R, 0, 1), packed like the
    kernel's block windows: [128 s', HC, total span cols]."""
    sp = np.arange(128, dtype=np.float32)[:, None]
    cols = []
    for tch in range(NT):
        for k in range(ST):
            tlo = t_lo(k, tch, span_full)
            m_w = span_width(k, tch, span_full)
            if m_w - tlo <= 0:
                continue
            delta = 128 * k - 512 * tch
            tp = np.arange(tlo, m_w, dtype=np.float32)[None, :]
            d = delta + sp - tp
            cols.append(np.where(d < 0, 0.0, d))
    if not cols:
        return np.zeros((128, HC, 1), np.float16)
    dall = np.concatenate(cols, axis=1)  # [128, S]
    mask = np.clip((R + z_heads[None, :, None] - dall[:, None, :]) / R,
                   0.0, 1.0)
    return mask.astype(np.float16)


def kernel(x, Wq, Wk, Wv, Wo, bo, Wspan, bspan):
    x = np.asarray(x, np.float32)
    Wq = np.asarray(Wq, np.float32)
    Wk = np.asarray(Wk, np.float32)
    Wv = np.asarray(Wv, np.float32)
    Wo = np.asarray(Wo, np.float32)
    bo = np.asarray(bo, np.float32)
    Wspan = np.asarray(Wspan, np.float32)
    bspan = np.asarray(bspan, np.float32)

    # span-mask restriction is only exact when z in [Z_MIN+6, Z_MAX-6]
    logits = x.mean(axis=1) @ Wspan.T + bspan
    z = T / (1.0 + np.exp(-logits))
    span_full = bool(z.min() < Z_MIN + 6.0 or z.max() > Z_MAX - 6.0)
    nc = build_nc(span_full=span_full)
    in_maps = _prep_core_inputs(x, Wq, Wk, Wv, Wo, Wspan, bspan,
                                span_full, _make_c01())
    res = run_bass_kernel_spmd(nc, in_maps, core_ids=list(range(N_CORES)))
    y = np.empty((B, T, D), np.float32)
    for b in range(B):
        y[b] = (res.results[2 * b]["yp"].astype(np.float32)
                + res.results[2 * b + 1]["yp"].astype(np.float32) + bo)
    return y


# revision 63
# speedup vs baseline: 1.1890x; 1.0031x over previous
"""AdaptiveSpanAttention Trainium2 kernel (8 NeuronCores).

Sharding: core c -> (batch b = c//2, head-group g = c%2).
Each core computes, for its batch and its 8 heads:
  Q/K/V projections in error-compensated fp8 DoubleRow (x and W split
  into fp8e4 hi+lo on host; the 3 significant cross products run with
  pair-slots packing two 128-k-tiles per pass -> 0.75 cycles/row vs
  bf16), anti-causal (j>=i) attention with adaptive-span mask in bf16,
  renormalization, and a partial bf16 output projection
  y_part = Out_g @ Wo[:, e_slice].T.
Host combines: y[b] = yp[2b] + yp[2b+1] + bo  (yp emitted as f16).

The span net (z = T*sigmoid(mean_t x @ WspanT + bspan)) and the full
adaptive-span masks clip((R + z - d)/R, 0, 1) are computed on host and
shipped as packed f16 tables, so the per-block mask application is a
single 2x-mode DVE multiply.

Projection weights are pre-scaled by SW=128 on host so the fp8 lo
residuals stay in e4m3's normal range; the inverse scales fold into
the exp scale and the renorm multiplier.

Scheduling notes: DMAs are batched (one per tensor, partition-major
host layout) because each DMA costs a serialized ~650ns issue slot;
dummy matmuls keep the PE p-state hot through the DMA lead-in; attnV
is software-pipelined one block behind the scores; the attention PSUM
pools hand their banks to the tail out-projections.
"""
import sys

sys.path.insert(0, "/opt/trn_rl_repo")

from contextlib import ExitStack

import ml_dtypes
import numpy as np

import concourse.bass as bass
import concourse.tile as tile
from concourse import bacc, mybir
from concourse.bass_utils import run_bass_kernel_spmd

BF16 = mybir.dt.bfloat16
F16 = mybir.dt.float16
FP8 = mybir.dt.float8e4
F32 = mybir.dt.float32
DR = mybir.MatmulPerfMode.DoubleRow

B, T, D, H = 4, 1024, 1024, 16
DH = 64          # head dim
R = 256.0
HC = 8           # heads per core
E = 512          # channels per core (HC * DH)
N_CORES = 8
TCH = 512        # t-chunk width (PSUM f32 free-dim limit)
NT = T // TCH    # 2 t-chunks
ST = T // 128    # 8 s-tiles
NM = 4           # contraction pair-tiles (1024 = 4 * 256)

SW = 128.0       # host pre-scale on projection weights
OS = 8.0         # attn-out pre-scale before its fp8 hi/lo split
EXP_SCALE = 1.0 / (8.0 * SW * SW)   # folds 1/sqrt(dh) and Q/K weight scales
Y_SCALE = 1.0 / (SW * OS)           # folds Wo and attn-out scales back out

_NC_CACHE = {}

# span-mask restriction bounds, verified on host per call (span_full
# fallback otherwise). z in [Z_MIN+6, Z_MAX-6] required.
Z_MIN = 490.0
Z_MAX = 545.0
CUT = int(R + Z_MAX)  # distance beyond which attention is exactly 0


def causal_width(st, tch):
    """Valid query-column width of block (s_tile=st, t_chunk=tch)."""
    delta = 128 * st - 512 * tch
    return max(0, min(TCH, delta + 128))


def span_width(st, tch, span_full):
    """Columns [0, m_w) where the span mask can differ from 1 (z >= Z_MIN)."""
    delta = 128 * st - 512 * tch
    w = causal_width(st, tch)
    if span_full:
        return w
    return max(0, min(w, delta + 127 - int(Z_MIN)))


def t_lo(st, tch, span_full):
    """Columns [0, t_lo) of the block are fully masked (dist >= R + z)."""
    if span_full:
        return 0
    delta = 128 * st - 512 * tch
    return max(0, delta - CUT)


def build_nc(span_full=False):
    key = ("nc", span_full)
    if key in _NC_CACHE:
        return _NC_CACHE[key]
    nc = bacc.Bacc("TRN2", target_bir_lowering=False, debug=False, num_devices=1)

    # ---- DRAM parameters (per-core shards, partition-major batched) ----
    # x pair tiles: [128 part][NM][2 slots][T]; slot i of pair tile m holds
    # xT rows [256m+128i, 256m+128(i+1)). Split into two halves (m 0-1, 2-3)
    # so the PE can start before the whole tensor lands.
    xh0_d = nc.declare_dram_parameter("xh0", [128, 2, 2, T], FP8, isOutput=False)
    xh1_d = nc.declare_dram_parameter("xh1", [128, 2, 2, T], FP8, isOutput=False)
    xl0_d = nc.declare_dram_parameter("xl0", [128, 2, 2, T], FP8, isOutput=False)
    xl1_d = nc.declare_dram_parameter("xl1", [128, 2, 2, T], FP8, isOutput=False)
    w_d = {}
    for wname in ("wq", "wk", "wv"):
        for lv in ("h", "l"):
            w_d[wname + lv] = nc.declare_dram_parameter(
                wname + lv, [128, NM, 2, E], FP8, isOutput=False)
    woT_d = nc.declare_dram_parameter("woT", [128, 4, D], BF16, isOutput=False)
    # packed span-ramp tiles (see _make_cneg); widths account for the
    # fully-masked column cut
    widths = [max(0, span_width(st, tc, span_full) - t_lo(st, tc, span_full))
              for tc in range(NT) for st in range(ST)]
    offs = np.concatenate([[0], np.cumsum(widths)]).astype(int)
    SMC = max(1, int(offs[-1]))
    smask_d = nc.declare_dram_parameter("smask", [128, HC, SMC], F16,
                                        isOutput=False)
    # cmask[:, k] for k<4: causal 0/1 multiplier (1 where s' >= j)
    cmask_d = nc.declare_dram_parameter("cmask", [128, 5, 128], BF16,
                                        isOutput=False)
    yp_d = nc.declare_dram_parameter("yp", [T, D], F16, isOutput=True)

    with tile.TileContext(nc) as tc, ExitStack() as ctx:
        # ---------------- pools ----------------
        consts = ctx.enter_context(tc.tile_pool(name="consts", bufs=1))
        xp = ctx.enter_context(tc.tile_pool(name="xp", bufs=1))
        wp = ctx.enter_context(tc.tile_pool(name="wp", bufs=1))
        qkp = ctx.enter_context(tc.tile_pool(name="qkp", bufs=1))
        vp = ctx.enter_context(tc.tile_pool(name="vp", bufs=1))
        outp = ctx.enter_context(tc.tile_pool(name="outp", bufs=1))
        scr = ctx.enter_context(tc.tile_pool(name="scr", bufs=3))
        ysb = ctx.enter_context(tc.tile_pool(name="ysb", bufs=4))

        lead_ctx = ExitStack()
        ps_lead = lead_ctx.enter_context(
            tc.tile_pool(name="ps_lead", bufs=7, space="PSUM"))
        ps_warm = lead_ctx.enter_context(
            tc.tile_pool(name="ps_warm", bufs=1, space="PSUM"))

        # ---------------- PE p-state warmup ----------------
        # The PE clock ramps with sustained use and resets on idle gaps.
        # Dummy matmuls on a zeroed tile keep it hot through the DMA lead-in.
        warm = consts.tile([128, TCH], BF16)
        nc.vector.memset(warm[:, 0:128], 0.0)
        nc.gpsimd.memset(warm[:, 128:TCH], 0.0)
        wps_holder = [None]

        def dummy(n=1, ap=TCH):
            if wps_holder[0] is None:
                wps_holder[0] = ps_warm.tile([128, TCH], F32, tag="warm",
                                             name="warmps")
            for _ in range(n):
                nc.tensor.matmul(wps_holder[0][:, 0:ap], warm[:, 0:128],
                                 warm[:, 0:ap], start=True, stop=True)

        dummy(3, ap=128)
        dummy(5)

        # ---------------- batched DMA loads ----------------
        xh_sb = xp.tile([128, 2, 2, 2, T], FP8, name="xh_sb")
        xl_sb = xp.tile([128, 2, 2, 2, T], FP8, name="xl_sb")
        nc.sync.dma_start(xh_sb[:, 0], xh0_d[:, :, :, :])
        wq_h = wp.tile([128, NM, 2, E], FP8, name="wq_h")
        nc.sync.dma_start(wq_h[:], w_d["wqh"][:, :, :, :])
        wk_h = wp.tile([128, NM, 2, E], FP8, name="wk_h")
        nc.sync.dma_start(wk_h[:], w_d["wkh"][:, :, :, :])
        nc.sync.dma_start(xh_sb[:, 1], xh1_d[:, :, :, :])
        nc.sync.dma_start(xl_sb[:, 0], xl0_d[:, :, :, :])
        nc.sync.dma_start(xl_sb[:, 1], xl1_d[:, :, :, :])
        wq_l = wp.tile([128, NM, 2, E], FP8, name="wq_l")
        nc.sync.dma_start(wq_l[:], w_d["wql"][:, :, :, :])
        wk_l = wp.tile([128, NM, 2, E], FP8, name="wk_l")
        nc.sync.dma_start(wk_l[:], w_d["wkl"][:, :, :, :])
        cmask_sb = consts.tile([128, 5, 128], BF16, name="cmask_sb")
        nc.sync.dma_start(cmask_sb[:], cmask_d[:, :, :])
        wv_h = wp.tile([128, NM, 2, E], FP8, name="wv_h")
        nc.sync.dma_start(wv_h[:], w_d["wvh"][:, :, :, :])
        wv_l = wp.tile([128, NM, 2, E], FP8, name="wv_l")
        nc.sync.dma_start(wv_l[:], w_d["wvl"][:, :, :, :])
        # span masks split per head pair so pair (0,0) unblocks early
        smask_sb = consts.tile([128, HC, SMC], F16, tag="smask")
        for jp2 in range(4):
            nc.sync.dma_start(smask_sb[:, 2 * jp2:2 * (jp2 + 1), :],
                              smask_d[:, 2 * jp2:2 * (jp2 + 1), :])
        wo_sb = wp.tile([128, 4, D], BF16, name="wo_sb")
        nc.sync.dma_start(wo_sb[:], woT_d[:, :, :])

        def xm(hi, m):
            t_ = xh_sb if hi else xl_sb
            return t_[:, m // 2, m % 2]

        wsb = {"wqh": wq_h, "wql": wq_l, "wkh": wk_h, "wkl": wk_l,
               "wvh": wv_h, "wvl": wv_l}

        # ---------------- Q/K projections (transposed layout) ----------------
        # QT[e, t] = sum_d WqT'[d, e] * xT[d, t] in compensated fp8.
        # Per pair-tile m the 3 products (hi.hi, lo_w.hi_x, hi_w.lo_x) run as
        # DoubleRow passes; lead groups are emitted m-major so the PE chases
        # the DMA stream.
        qt_sb = [qkp.tile([128, T], BF16, tag="qt", name=f"qt{i}", bufs=4)
                 for i in range(4)]
        kt_sb = [qkp.tile([128, T], BF16, tag="kt", name=f"kt{i}", bufs=4)
                 for i in range(4)]

        def qk_mm(ps, wn, et, tch, m, prod, first=False, last=False):
            """One product matmul: prod 0 = hi.hi, 1 = hi_w.lo_x,
            2 = lo_w.hi_x."""
            es = slice(128 * et, 128 * (et + 1))
            ts = slice(TCH * tch, TCH * (tch + 1))
            w_t = wsb[wn + ("h" if prod < 2 else "l")][:, m]
            x_t = xm(1 if prod != 1 else 0, m)
            nc.tensor.matmul(
                ps[:], w_t[:, :, es], x_t[:, :, ts],
                start=first, stop=last, perf_mode=DR, skip_group_check=True)

        def qk_copy(dst_sb, et, tch, ps, eng="act"):
            ts = slice(TCH * tch, TCH * (tch + 1))
            if eng == "act":
                nc.scalar.copy(dst_sb[et][:, ts], ps[:])
            else:
                nc.vector.tensor_copy(dst_sb[et][:, ts], ps[:])

        # 7 lead groups chase the DMA stream in availability order:
        # all hi.hi products (x_hi + W_hi land first), then hi_w.lo_x
        # (x_lo next), then lo_w.hi_x (W_lo last)
        lead_defs = [
            (qt_sb, "wq", 0, 0), (kt_sb, "wk", 0, 0),
            (qt_sb, "wq", 1, 0), (kt_sb, "wk", 1, 0),
            (qt_sb, "wq", 0, 1), (kt_sb, "wk", 0, 1),
            (qt_sb, "wq", 1, 1),
        ]
        lead_ps = [ps_lead.tile([128, TCH], F32, tag="pj", name=f"pl{i}")
                   for i in range(len(lead_defs))]
        for prod in range(2):
            for m in range(NM):
                for gi, (dst, wn, et, tch) in enumerate(lead_defs):
                    qk_mm(lead_ps[gi], wn, et, tch, m, prod,
                          first=(prod == 0 and m == 0))
                if prod == 0:
                    dummy(3 if m >= 2 else 2)
        # finish group-by-group so qt0/kt0 unlock the attention start early
        for gi, (dst, wn, et, tch) in enumerate(lead_defs):
            for m in range(NM):
                qk_mm(lead_ps[gi], wn, et, tch, m, 2, last=(m == NM - 1))
            qk_copy(dst, et, tch, lead_ps[gi])

        def emit_qk(dst_sb, wn, et, tch, pool):
            ps = pool.tile([128, TCH], F32, tag="pj", name=f"pj{et}_{tch}")
            for m in range(NM):
                for prod in range(3):
                    qk_mm(ps, wn, et, tch, m, prod,
                          first=(m == 0 and prod == 0),
                          last=(m == NM - 1 and prod == 2))
            qk_copy(dst_sb, et, tch, ps, eng="dve")

        # ---------------- V (natural layout, ones-augmented) ----------------
        v_aug = [None] * ST

        def emit_v(st, pool):
            va = vp.tile([128, HC, 2 * DH], BF16, tag="vaug", bufs=ST,
                         name=f"vaug{st}")
            nc.gpsimd.memset(va[:, :, DH:2 * DH], 1.0)
            ps = pool.tile([128, E], F32, tag="pj", name=f"pjv{st}",
                           padded_shape=[128, TCH])
            ss = slice(128 * st, 128 * (st + 1))
            ops = ([(xm(1, m), wv_h[:, m]) for m in range(NM)]
                   + [(xm(1, m), wv_l[:, m]) for m in range(NM)]
                   + [(xm(0, m), wv_h[:, m]) for m in range(NM)])
            for i, (x_t, w_t) in enumerate(ops):
                nc.tensor.matmul(
                    ps[:], x_t[:, :, ss], w_t[:],
                    start=(i == 0), stop=(i == len(ops) - 1), perf_mode=DR,
                    skip_group_check=True)
            nc.vector.tensor_copy(
                va[:, :, 0:DH], ps[:].rearrange("p (h d) -> p h d", h=HC))
            v_aug[st] = va

        emit_qk(kt_sb, "wk", 1, 1, ps_lead)
        emit_v(0, ps_lead)
        emit_v(1, ps_lead)

        lead_ctx.close()
        ps_proj = ctx.enter_context(tc.tile_pool(name="ps_proj", bufs=2, space="PSUM"))
        attn_ctx = ExitStack()
        ps_sc = attn_ctx.enter_context(
            tc.tile_pool(name="ps_sc", bufs=2, space="PSUM"))
        ps_out = attn_ctx.enter_context(
            tc.tile_pool(name="ps_out", bufs=2, space="PSUM"))

        # ---------------- attention ----------------
        # out_pair[j][tch] holds heads 2j (parts 0:64) and 2j+1 (parts 64:128)
        out_pair = [[outp.tile([128, TCH], BF16, tag="out", bufs=8,
                               name=f"op{j}_{c}") for c in range(NT)]
                    for j in range(4)]

        def attn_pair(tch, j, v_prefetch=False, fillers=(), renorm_tts=1):
            """Attention for head pair (2j, 2j+1); both share et=j."""
            first_st = 4 * tch
            heads = (2 * j, 2 * j + 1)
            pouts = [ps_out.tile([128, TCH], F32, tag="pout",
                                 name=f"pout{h}_{tch}") for h in heads]

            def emit_av(st, tlo, w, p_hp):
                for i, h in enumerate(heads):
                    nc.tensor.matmul(
                        pouts[i][:, tlo:w], v_aug[st][:, h, :],
                        p_hp[:, i, tlo:w],
                        start=(st == first_st), stop=(st == ST - 1),
                        skip_group_check=True)

            pending = None
            fillers = list(fillers)
            for st in range(first_st, ST):
                if v_prefetch:
                    if st == first_st and v_aug[st + 2] is None:
                        emit_v(st + 2, ps_proj)
                    if st + 3 < ST and v_aug[st + 3] is None:
                        emit_v(st + 3, ps_proj)
                if fillers:
                    f = fillers.pop(0)
                    if f is not None:
                        f()
                w = causal_width(st, tch)
                k = st - first_st  # delta = 128*k
                tlo = t_lo(st, tch, span_full)
                m_w = span_width(st, tch, span_full)
                moff = offs[8 * tch + st]
                sc_hp = ps_sc.tile([128, 2, TCH], F32, tag="sc",
                                   name=f"sc{j}_{st}")
                diag = k <= 3
                for i, h in enumerate(heads):
                    hp = (h % 2) * 64
                    nc.tensor.matmul(
                        sc_hp[:, i, tlo:w],
                        kt_sb[j][hp:hp + DH, 128 * st:128 * (st + 1)],
                        qt_sb[j][hp:hp + DH, TCH * tch + tlo:TCH * tch + w],
                        start=True, stop=True, skip_group_check=True)
                p_hp = scr.tile([128, 2, TCH], BF16, tag="p", bufs=10,
                                name=f"p{j}_{st}")
                nc.scalar.activation(
                    p_hp[:, :, tlo:w], sc_hp[:, :, tlo:w],
                    mybir.ActivationFunctionType.Exp, scale=EXP_SCALE)
                if diag:
                    # causal zeroing of the diagonal 128x128 sub-block
                    d0 = 128 * k
                    for i, h in enumerate(heads):
                        ceng = nc.vector if tch == 1 else nc.gpsimd
                        ceng.tensor_mul(
                            p_hp[:, i, d0:w], p_hp[:, i, d0:w],
                            cmask_sb[:, k, 0:w - d0])
                if m_w > tlo:
                    for i, h in enumerate(heads):
                        # span mask precomputed on host: one 2x-mode multiply
                        nc.vector.tensor_mul(
                            p_hp[:, i, tlo:m_w], p_hp[:, i, tlo:m_w],
                            smask_sb[:, h, moff:moff + m_w - tlo])
                # software pipeline: attnV for the PREVIOUS block runs now,
                # so it never waits on this block's exp/mask chain
                if pending is not None:
                    emit_av(*pending)
                pending = (st, tlo, w, p_hp)
            emit_av(*pending)
            # rows 0:64 numerator (scaled SW); rows 64:128 denominator W
            nchunk = TCH // renorm_tts
            for rchunk in range(renorm_tts):
                cs = slice(rchunk * nchunk, (rchunk + 1) * nchunk)
                for i, h in enumerate(heads):
                    hp = (h % 2) * 64
                    pout = pouts[i]
                    rw = scr.tile([DH, TCH], F32, tag="rw", bufs=4,
                                  name=f"rw{h}_{rchunk}")
                    with nc.allow_low_precision(reason="denom recip"):
                        nc.vector.reciprocal(rw[:, cs], pout[DH:2 * DH, cs])
                    nc.vector.scalar_tensor_tensor(
                        out_pair[j][tch][hp:hp + DH, cs], pout[0:DH, cs],
                        1.0 / SW, rw[:, cs],
                        op0=mybir.AluOpType.mult, op1=mybir.AluOpType.mult)

        def op_matmuls(yps, tch, toff, ns):
            for j in range(4):
                nc.tensor.matmul(
                    yps[:], out_pair[j][tch][:, toff:toff + 128],
                    wo_sb[:, j, ns],
                    start=(j == 0), stop=(j == 3), skip_group_check=True)

        def yo_copy(yo, ns, yps, eng):
            with nc.allow_low_precision(reason="f16 output"):
                if eng % 2 == 0:
                    nc.scalar.copy(yo[:, ns], yps[:])
                else:
                    nc.vector.tensor_copy(yo[:, ns], yps[:])

        def op_tt(tt, split_dma=False, pool=None):
            """Out-projection for t-tile tt: both 512-chunks + DMA out."""
            pool = pool or ps_proj
            tch = tt // 4
            toff = 128 * tt - TCH * tch
            yo = ysb.tile([128, D], F16, tag="y")
            for nch in range(NT):
                ns = slice(TCH * nch, TCH * (nch + 1))
                yps = pool.tile([128, TCH], F32, tag="pj",
                                name=f"y{tt}_{nch}")
                op_matmuls(yps, tch, toff, ns)
                # the final tiles' copies go on the fast engines (ACT/DVE)
                # and issue their DMA from the ACT queue (skips the busy
                # SP queue at the very end of the kernel)
                eng = nch if tt >= 6 else (tt + nch) % 3
                yo_copy(yo, ns, yps, eng)
                if split_dma:
                    deng = nc.scalar if eng == 0 else nc.sync
                    deng.dma_start(yp_d[128 * tt:128 * (tt + 1), ns],
                                   yo[:, ns])
            if not split_dma:
                nc.sync.dma_start(yp_d[128 * tt:128 * (tt + 1), :], yo[:])

        qk_rest = [(qt_sb, "wq", 2, 0), (kt_sb, "wk", 2, 0),
                   (qt_sb, "wq", 3, 0), (kt_sb, "wk", 3, 0),
                   (qt_sb, "wq", 2, 1), (kt_sb, "wk", 2, 1),
                   (qt_sb, "wq", 3, 1), (kt_sb, "wk", 3, 1)]

        def qk_f(idx):
            dst, wn, et, tch = qk_rest[idx]
            return lambda: emit_qk(dst, wn, et, tch, ps_proj)

        attn_pair(0, 0, v_prefetch=True,
                  fillers=[qk_f(0), None, qk_f(1), None])
        attn_pair(0, 1, fillers=[qk_f(2), qk_f(3)])
        attn_pair(0, 2, fillers=[qk_f(4), qk_f(5)])
        attn_pair(0, 3, fillers=[qk_f(6), qk_f(7)])
        def op_nch(tt, nch):
            """Half of op_tt as a filler unit; DMA fires on the second half."""
            tch = tt // 4
            toff = 128 * tt - TCH * tch
            ns = slice(TCH * nch, TCH * (nch + 1))
            if nch == 0:
                op_nch.yo[tt] = ysb.tile([128, D], F16, tag="y",
                                         name=f"yon{tt}")
            yo = op_nch.yo[tt]
            yps = ps_proj.tile([128, TCH], F32, tag="pj", name=f"y{tt}_{nch}")
            op_matmuls(yps, tch, toff, ns)
            yo_copy(yo, ns, yps, (tt + nch) % 3)
            if nch == NT - 1:
                nc.sync.dma_start(yp_d[128 * tt:128 * (tt + 1), :], yo[:])
        op_nch.yo = {}

        attn_pair(1, 0, fillers=[lambda: op_nch(0, 0), lambda: op_nch(0, 1)])
        attn_pair(1, 1, fillers=[lambda: op_nch(1, 0), lambda: op_nch(1, 1)])
        attn_pair(1, 2, fillers=[lambda: op_nch(2, 0), lambda: op_nch(2, 1)])
        attn_pair(1, 3, fillers=[lambda: op_nch(3, 0), None,
                                 lambda: dummy(3, ap=256)], renorm_tts=4)
        op_nch(3, 1)
        # attention PSUM pools are done; hand their banks to the tail
        # out-projections so four groups can be in flight
        attn_ctx.close()
        ps_tail = ctx.enter_context(
            tc.tile_pool(name="ps_tail", bufs=4, space="PSUM"))
        for tt in range(4, 8):
            op_tt(tt, pool=ps_tail)

    nc.compile()
    _NC_CACHE[key] = nc
    return nc


def _f8pair(a):
    f8 = ml_dtypes.float8_e4m3fn
    hi = a.astype(f8)
    lo = (a - hi.astype(np.float32)).astype(f8)
    return hi, lo


def _pm_pairtiles(a):
    """[Ktot, N] -> partition-major [128, Ktot/256, 2, N]:
    out[p, m, i, :] = a[256m + 128i + p, :]."""
    K = a.shape[0]
    rest = a.shape[1:]
    return np.ascontiguousarray(
        a.reshape(K // 256, 2, 128, *rest).transpose(2, 0, 1, 3))


def _prep_core_inputs(x, Wq, Wk, Wv, Wo, Wspan, bspan, span_full, cmask):
    bf = ml_dtypes.bfloat16
    in_maps = []
    # span net on host (f32): z per (batch, head)
    logits = x.mean(axis=1) @ Wspan.T + bspan
    z = T / (1.0 + np.exp(-logits))
    for c in range(N_CORES):
        b, g = c // 2, c % 2
        hs = slice(E * g, E * (g + 1))
        xt = np.ascontiguousarray(x[b].T).astype(np.float32)
        xh, xl = _f8pair(xt)
        xhp = _pm_pairtiles(xh)
        xlp = _pm_pairtiles(xl)
        m = {
            "cmask": cmask,
            "xh0": np.ascontiguousarray(xhp[:, 0:2]),
            "xh1": np.ascontiguousarray(xhp[:, 2:4]),
            "xl0": np.ascontiguousarray(xlp[:, 0:2]),
            "xl1": np.ascontiguousarray(xlp[:, 2:4]),
            "smask": _make_smask(z[b, HC * g:HC * (g + 1)], span_full),
        }
        m["woT"] = np.ascontiguousarray(
            Wo[:, hs].T.reshape(4, 128, D).transpose(1, 0, 2)).astype(bf)
        for wname, W in (("wq", Wq), ("wk", Wk), ("wv", Wv)):
            wt = np.ascontiguousarray(W[hs, :].T).astype(np.float32) * SW
            wh, wl = _f8pair(wt)
            m[wname + "h"] = _pm_pairtiles(wh)
            m[wname + "l"] = _pm_pairtiles(wl)
        in_maps.append(m)
    return in_maps


def _make_c01():
    """cmask: [:, 0:4] causal 0/1 (1 where s' >= j), [:, 4] identity."""
    sp = np.arange(128, dtype=np.float32)[:, None]
    jp = np.arange(128, dtype=np.float32)[None, :]
    c01 = (sp - jp >= 0).astype(np.float32)
    ident = np.eye(128, dtype=np.float32)
    stk = np.stack([c01, c01, c01, c01, ident])  # [5, 128, 128]
    return np.ascontiguousarray(stk.transpose(1, 0, 2)).astype(
        ml_dtypes.bfloat16)


def _make_smask(z_heads, span_full):
    """Per-head span masks clip((R + z - d)/R, 0, 1), packed like the
    kernel's block windows: [128 s', HC, total span cols]."""
    sp = np.arange(128, dtype=np.float32)[:, None]
    cols = []
    for tch in range(NT):
        for k in range(ST):
            tlo = t_lo(k, tch, span_full)
            m_w = span_width(k, tch, span_full)
            if m_w - tlo <= 0:
                continue
            delta = 128 * k - 512 * tch
            tp = np.arange(tlo, m_w, dtype=np.float32)[None, :]
            d = delta + sp - tp
            cols.append(np.where(d < 0, 0.0, d))
    if not cols:
        return np.zeros((128, HC, 1), np.float16)
    dall = np.concatenate(cols, axis=1)  # [128, S]
    mask = np.clip((R + z_heads[None, :, None] - dall[:, None, :]) / R,
                   0.0, 1.0)
    return mask.astype(np.float16)


def kernel(x, Wq, Wk, Wv, Wo, bo, Wspan, bspan):
    x = np.asarray(x, np.float32)
    Wq = np.asarray(Wq, np.float32)
    Wk = np.asarray(Wk, np.float32)
    Wv = np.asarray(Wv, np.float32)
    Wo = np.asarray(Wo, np.float32)
    bo = np.asarray(bo, np.float32)
    Wspan = np.asarray(Wspan, np.float32)
    bspan = np.asarray(bspan, np.float32)

    # span-mask restriction is only exact when z in [Z_MIN+6, Z_MAX-6]
    logits = x.mean(axis=1) @ Wspan.T + bspan
    z = T / (1.0 + np.exp(-logits))
    span_full = bool(z.min() < Z_MIN + 6.0 or z.max() > Z_MAX - 6.0)
    nc = build_nc(span_full=span_full)
    in_maps = _prep_core_inputs(x, Wq, Wk, Wv, Wo, Wspan, bspan,
                                span_full, _make_c01())
    res = run_bass_kernel_spmd(nc, in_maps, core_ids=list(range(N_CORES)))
    y = np.empty((B, T, D), np.float32)
    for b in range(B):
        y[b] = (res.results[2 * b]["yp"].astype(np.float32)
                + res.results[2 * b + 1]["yp"].astype(np.float32) + bo)
    return y


# revision 68
# speedup vs baseline: 1.1907x; 1.0015x over previous
"""AdaptiveSpanAttention Trainium2 kernel (8 NeuronCores).

Sharding: core c -> (batch b = c//2, head-group g = c%2).
Each core computes, for its batch and its 8 heads:
  Q/K/V projections in error-compensated fp8 DoubleRow (x and W split
  into fp8e4 hi+lo on host; the 3 significant cross products run with
  pair-slots packing two 128-k-tiles per pass -> 0.75 cycles/row vs
  bf16), anti-causal (j>=i) attention with adaptive-span mask in bf16,
  renormalization, and a partial bf16 output projection
  y_part = Out_g @ Wo[:, e_slice].T.
Host combines: y[b] = yp[2b] + yp[2b+1] + bo  (yp emitted as f16).

The span net (z = T*sigmoid(mean_t x @ WspanT + bspan)) and the full
adaptive-span masks clip((R + z - d)/R, 0, 1) are computed on host and
shipped as packed f16 tables, so the per-block mask application is a
single 2x-mode DVE multiply.

Projection weights are pre-scaled by SW=128 on host so the fp8 lo
residuals stay in e4m3's normal range; the inverse scales fold into
the exp scale and the renorm multiplier.

Scheduling notes: DMAs are batched (one per tensor, partition-major
host layout) because each DMA costs a serialized ~650ns issue slot;
dummy matmuls keep the PE p-state hot through the DMA lead-in; attnV
is software-pipelined one block behind the scores; the attention PSUM
pools hand their banks to the tail out-projections.
"""
import sys

sys.path.insert(0, "/opt/trn_rl_repo")

from contextlib import ExitStack

import ml_dtypes
import numpy as np

import concourse.bass as bass
import concourse.tile as tile
from concourse import bacc, mybir
from concourse.bass_utils import run_bass_kernel_spmd

BF16 = mybir.dt.bfloat16
F16 = mybir.dt.float16
FP8 = mybir.dt.float8e4
F32 = mybir.dt.float32
DR = mybir.MatmulPerfMode.DoubleRow

B, T, D, H = 4, 1024, 1024, 16
DH = 64          # head dim
R = 256.0
HC = 8           # heads per core
E = 512          # channels per core (HC * DH)
N_CORES = 8
TCH = 512        # t-chunk width (PSUM f32 free-dim limit)
NT = T // TCH    # 2 t-chunks
ST = T // 128    # 8 s-tiles
NM = 4           # contraction pair-tiles (1024 = 4 * 256)

SW = 128.0       # host pre-scale on projection weights
OS = 8.0         # attn-out pre-scale before its fp8 hi/lo split
EXP_SCALE = 1.0 / (8.0 * SW * SW)   # folds 1/sqrt(dh) and Q/K weight scales
Y_SCALE = 1.0 / (SW * OS)           # folds Wo and attn-out scales back out

_NC_CACHE = {}

# span-mask restriction bounds, verified on host per call (span_full
# fallback otherwise). z in [Z_MIN+6, Z_MAX-6] required.
Z_MIN = 490.0
Z_MAX = 545.0
CUT = int(R + Z_MAX)  # distance beyond which attention is exactly 0


def causal_width(st, tch):
    """Valid query-column width of block (s_tile=st, t_chunk=tch)."""
    delta = 128 * st - 512 * tch
    return max(0, min(TCH, delta + 128))


def span_width(st, tch, span_full):
    """Columns [0, m_w) where the span mask can differ from 1 (z >= Z_MIN)."""
    delta = 128 * st - 512 * tch
    w = causal_width(st, tch)
    if span_full:
        return w
    return max(0, min(w, delta + 127 - int(Z_MIN)))


def t_lo(st, tch, span_full):
    """Columns [0, t_lo) of the block are fully masked (dist >= R + z)."""
    if span_full:
        return 0
    delta = 128 * st - 512 * tch
    return max(0, delta - CUT)


def build_nc(span_full=False):
    key = ("nc", span_full)
    if key in _NC_CACHE:
        return _NC_CACHE[key]
    nc = bacc.Bacc("TRN2", target_bir_lowering=False, debug=False, num_devices=1)

    # ---- DRAM parameters (per-core shards, partition-major batched) ----
    # x pair tiles: [128 part][NM][2 slots][T]; slot i of pair tile m holds
    # xT rows [256m+128i, 256m+128(i+1)). Split into two halves (m 0-1, 2-3)
    # so the PE can start before the whole tensor lands.
    xh0_d = nc.declare_dram_parameter("xh0", [128, 2, 2, T], FP8, isOutput=False)
    xh1_d = nc.declare_dram_parameter("xh1", [128, 2, 2, T], FP8, isOutput=False)
    xl0_d = nc.declare_dram_parameter("xl0", [128, 2, 2, T], FP8, isOutput=False)
    xl1_d = nc.declare_dram_parameter("xl1", [128, 2, 2, T], FP8, isOutput=False)
    w_d = {}
    for wname in ("wq", "wk", "wv"):
        for lv in ("h", "l"):
            w_d[wname + lv] = nc.declare_dram_parameter(
                wname + lv, [128, NM, 2, E], FP8, isOutput=False)
    woT_d = nc.declare_dram_parameter("woT", [128, 4, D], BF16, isOutput=False)
    # packed span-ramp tiles (see _make_cneg); widths account for the
    # fully-masked column cut
    widths = [max(0, span_width(st, tc, span_full) - t_lo(st, tc, span_full))
              for tc in range(NT) for st in range(ST)]
    offs = np.concatenate([[0], np.cumsum(widths)]).astype(int)
    SMC = max(1, int(offs[-1]))
    smask_d = nc.declare_dram_parameter("smask", [128, HC, SMC], F16,
                                        isOutput=False)
    # cmask[:, k] for k<4: causal 0/1 multiplier (1 where s' >= j)
    cmask_d = nc.declare_dram_parameter("cmask", [128, 5, 128], BF16,
                                        isOutput=False)
    yp_d = nc.declare_dram_parameter("yp", [T, D], F16, isOutput=True)

    with tile.TileContext(nc) as tc, ExitStack() as ctx:
        # ---------------- pools ----------------
        consts = ctx.enter_context(tc.tile_pool(name="consts", bufs=1))
        xp = ctx.enter_context(tc.tile_pool(name="xp", bufs=1))
        wp = ctx.enter_context(tc.tile_pool(name="wp", bufs=1))
        qkp = ctx.enter_context(tc.tile_pool(name="qkp", bufs=1))
        vp = ctx.enter_context(tc.tile_pool(name="vp", bufs=1))
        outp = ctx.enter_context(tc.tile_pool(name="outp", bufs=1))
        scr = ctx.enter_context(tc.tile_pool(name="scr", bufs=3))
        ysb = ctx.enter_context(tc.tile_pool(name="ysb", bufs=4))

        lead_ctx = ExitStack()
        ps_lead = lead_ctx.enter_context(
            tc.tile_pool(name="ps_lead", bufs=7, space="PSUM"))
        ps_warm = lead_ctx.enter_context(
            tc.tile_pool(name="ps_warm", bufs=1, space="PSUM"))

        # ---------------- PE p-state warmup ----------------
        # The PE clock ramps with sustained use and resets on idle gaps.
        # Dummy matmuls on a zeroed tile keep it hot through the DMA lead-in.
        warm = consts.tile([128, TCH], BF16)
        nc.vector.memset(warm[:, 0:128], 0.0)
        nc.gpsimd.memset(warm[:, 128:TCH], 0.0)
        wps_holder = [None]

        def dummy(n=1, ap=TCH):
            if wps_holder[0] is None:
                wps_holder[0] = ps_warm.tile([128, TCH], F32, tag="warm",
                                             name="warmps")
            for _ in range(n):
                nc.tensor.matmul(wps_holder[0][:, 0:ap], warm[:, 0:128],
                                 warm[:, 0:ap], start=True, stop=True)

        dummy(3, ap=128)
        dummy(5)

        # ---------------- batched DMA loads ----------------
        xh_sb = xp.tile([128, 2, 2, 2, T], FP8, name="xh_sb")
        xl_sb = xp.tile([128, 2, 2, 2, T], FP8, name="xl_sb")
        nc.sync.dma_start(xh_sb[:, 0], xh0_d[:, :, :, :])
        wq_h = wp.tile([128, NM, 2, E], FP8, name="wq_h")
        nc.sync.dma_start(wq_h[:], w_d["wqh"][:, :, :, :])
        wk_h = wp.tile([128, NM, 2, E], FP8, name="wk_h")
        nc.sync.dma_start(wk_h[:], w_d["wkh"][:, :, :, :])
        nc.sync.dma_start(xh_sb[:, 1], xh1_d[:, :, :, :])
        nc.sync.dma_start(xl_sb[:, 0], xl0_d[:, :, :, :])
        nc.sync.dma_start(xl_sb[:, 1], xl1_d[:, :, :, :])
        wq_l = wp.tile([128, NM, 2, E], FP8, name="wq_l")
        nc.sync.dma_start(wq_l[:], w_d["wql"][:, :, :, :])
        wk_l = wp.tile([128, NM, 2, E], FP8, name="wk_l")
        nc.sync.dma_start(wk_l[:], w_d["wkl"][:, :, :, :])
        cmask_sb = consts.tile([128, 5, 128], BF16, name="cmask_sb")
        nc.sync.dma_start(cmask_sb[:], cmask_d[:, :, :])
        wv_h = wp.tile([128, NM, 2, E], FP8, name="wv_h")
        nc.sync.dma_start(wv_h[:], w_d["wvh"][:, :, :, :])
        wv_l = wp.tile([128, NM, 2, E], FP8, name="wv_l")
        nc.sync.dma_start(wv_l[:], w_d["wvl"][:, :, :, :])
        # span masks split per head pair so pair (0,0) unblocks early
        smask_sb = consts.tile([128, HC, SMC], F16, tag="smask")
        for jp2 in range(4):
            nc.sync.dma_start(smask_sb[:, 2 * jp2:2 * (jp2 + 1), :],
                              smask_d[:, 2 * jp2:2 * (jp2 + 1), :])
        wo_sb = wp.tile([128, 4, D], BF16, name="wo_sb")
        nc.sync.dma_start(wo_sb[:], woT_d[:, :, :])

        def xm(hi, m):
            t_ = xh_sb if hi else xl_sb
            return t_[:, m // 2, m % 2]

        wsb = {"wqh": wq_h, "wql": wq_l, "wkh": wk_h, "wkl": wk_l,
               "wvh": wv_h, "wvl": wv_l}

        # ---------------- Q/K projections (transposed layout) ----------------
        # QT[e, t] = sum_d WqT'[d, e] * xT[d, t] in compensated fp8.
        # Per pair-tile m the 3 products (hi.hi, lo_w.hi_x, hi_w.lo_x) run as
        # DoubleRow passes; lead groups are emitted m-major so the PE chases
        # the DMA stream.
        qt_sb = [qkp.tile([128, T], BF16, tag="qt", name=f"qt{i}", bufs=4)
                 for i in range(4)]
        kt_sb = [qkp.tile([128, T], BF16, tag="kt", name=f"kt{i}", bufs=4)
                 for i in range(4)]

        def qk_mm(ps, wn, et, tch, m, prod, first=False, last=False):
            """One product matmul: prod 0 = hi.hi, 1 = hi_w.lo_x,
            2 = lo_w.hi_x."""
            es = slice(128 * et, 128 * (et + 1))
            ts = slice(TCH * tch, TCH * (tch + 1))
            w_t = wsb[wn + ("h" if prod < 2 else "l")][:, m]
            x_t = xm(1 if prod != 1 else 0, m)
            nc.tensor.matmul(
                ps[:], w_t[:, :, es], x_t[:, :, ts],
                start=first, stop=last, perf_mode=DR, skip_group_check=True)

        def qk_copy(dst_sb, et, tch, ps, eng="act"):
            ts = slice(TCH * tch, TCH * (tch + 1))
            if eng == "act":
                nc.scalar.copy(dst_sb[et][:, ts], ps[:])
            else:
                nc.vector.tensor_copy(dst_sb[et][:, ts], ps[:])

        # 7 lead groups chase the DMA stream in availability order:
        # all hi.hi products (x_hi + W_hi land first), then hi_w.lo_x
        # (x_lo next), then lo_w.hi_x (W_lo last)
        lead_defs = [
            (qt_sb, "wq", 0, 0), (kt_sb, "wk", 0, 0),
            (qt_sb, "wq", 1, 0), (kt_sb, "wk", 1, 0),
            (qt_sb, "wq", 0, 1), (kt_sb, "wk", 0, 1),
            (qt_sb, "wq", 1, 1),
        ]
        lead_ps = [ps_lead.tile([128, TCH], F32, tag="pj", name=f"pl{i}")
                   for i in range(len(lead_defs))]
        for prod in range(2):
            for m in range(NM):
                for gi, (dst, wn, et, tch) in enumerate(lead_defs):
                    qk_mm(lead_ps[gi], wn, et, tch, m, prod,
                          first=(prod == 0 and m == 0))
                if prod == 0:
                    dummy(3 if m >= 2 else 2)
        # finish group-by-group so qt0/kt0 unlock the attention start early
        for gi, (dst, wn, et, tch) in enumerate(lead_defs):
            for m in range(NM):
                qk_mm(lead_ps[gi], wn, et, tch, m, 2, last=(m == NM - 1))
            qk_copy(dst, et, tch, lead_ps[gi])

        def emit_qk(dst_sb, wn, et, tch, pool):
            ps = pool.tile([128, TCH], F32, tag="pj", name=f"pj{et}_{tch}")
            for m in range(NM):
                for prod in range(3):
                    qk_mm(ps, wn, et, tch, m, prod,
                          first=(m == 0 and prod == 0),
                          last=(m == NM - 1 and prod == 2))
            qk_copy(dst_sb, et, tch, ps, eng="dve")

        # ---------------- V (natural layout, ones-augmented) ----------------
        v_aug = [None] * ST

        def emit_v(st, pool):
            va = vp.tile([128, HC, 2 * DH], BF16, tag="vaug", bufs=ST,
                         name=f"vaug{st}")
            nc.gpsimd.memset(va[:, :, DH:2 * DH], 1.0)
            ps = pool.tile([128, E], F32, tag="pj", name=f"pjv{st}",
                           padded_shape=[128, TCH])
            ss = slice(128 * st, 128 * (st + 1))
            ops = ([(xm(1, m), wv_h[:, m]) for m in range(NM)]
                   + [(xm(1, m), wv_l[:, m]) for m in range(NM)]
                   + [(xm(0, m), wv_h[:, m]) for m in range(NM)])
            for i, (x_t, w_t) in enumerate(ops):
                nc.tensor.matmul(
                    ps[:], x_t[:, :, ss], w_t[:],
                    start=(i == 0), stop=(i == len(ops) - 1), perf_mode=DR,
                    skip_group_check=True)
            nc.vector.tensor_copy(
                va[:, :, 0:DH], ps[:].rearrange("p (h d) -> p h d", h=HC))
            v_aug[st] = va

        emit_qk(kt_sb, "wk", 1, 1, ps_lead)
        emit_v(0, ps_lead)
        emit_v(1, ps_lead)

        lead_ctx.close()
        ps_proj = ctx.enter_context(tc.tile_pool(name="ps_proj", bufs=2, space="PSUM"))
        attn_ctx = ExitStack()
        ps_sc = attn_ctx.enter_context(
            tc.tile_pool(name="ps_sc", bufs=2, space="PSUM"))
        ps_out = attn_ctx.enter_context(
            tc.tile_pool(name="ps_out", bufs=2, space="PSUM"))

        # ---------------- attention ----------------
        # out_pair[j][tch] holds heads 2j (parts 0:64) and 2j+1 (parts 64:128)
        out_pair = [[outp.tile([128, TCH], BF16, tag="out", bufs=8,
                               name=f"op{j}_{c}") for c in range(NT)]
                    for j in range(4)]

        def attn_pair(tch, j, v_prefetch=False, fillers=(), renorm_tts=1):
            """Attention for head pair (2j, 2j+1); both share et=j."""
            first_st = 4 * tch
            heads = (2 * j, 2 * j + 1)
            pouts = [ps_out.tile([128, TCH], F32, tag="pout",
                                 name=f"pout{h}_{tch}") for h in heads]

            def emit_av(st, tlo, w, p_hp):
                for i, h in enumerate(heads):
                    nc.tensor.matmul(
                        pouts[i][:, tlo:w], v_aug[st][:, h, :],
                        p_hp[:, i, tlo:w],
                        start=(st == first_st), stop=(st == ST - 1),
                        skip_group_check=True)

            pending = None
            fillers = list(fillers)
            for st in range(first_st, ST):
                if v_prefetch:
                    if st == first_st and v_aug[st + 2] is None:
                        emit_v(st + 2, ps_proj)
                    if st + 3 < ST and v_aug[st + 3] is None:
                        emit_v(st + 3, ps_proj)
                if fillers:
                    f = fillers.pop(0)
                    if f is not None:
                        f()
                w = causal_width(st, tch)
                k = st - first_st  # delta = 128*k
                tlo = t_lo(st, tch, span_full)
                m_w = span_width(st, tch, span_full)
                moff = offs[8 * tch + st]
                sc_hp = ps_sc.tile([128, 2, TCH], F32, tag="sc",
                                   name=f"sc{j}_{st}")
                diag = k <= 3
                for i, h in enumerate(heads):
                    hp = (h % 2) * 64
                    nc.tensor.matmul(
                        sc_hp[:, i, tlo:w],
                        kt_sb[j][hp:hp + DH, 128 * st:128 * (st + 1)],
                        qt_sb[j][hp:hp + DH, TCH * tch + tlo:TCH * tch + w],
                        start=True, stop=True, skip_group_check=True)
                p_hp = scr.tile([128, 2, TCH], BF16, tag="p", bufs=10,
                                name=f"p{j}_{st}")
                nc.scalar.activation(
                    p_hp[:, :, tlo:w], sc_hp[:, :, tlo:w],
                    mybir.ActivationFunctionType.Exp, scale=EXP_SCALE)
                if diag:
                    # causal zeroing of the diagonal 128x128 sub-block
                    d0 = 128 * k
                    for i, h in enumerate(heads):
                        ceng = nc.vector if tch == 1 else nc.gpsimd
                        ceng.tensor_mul(
                            p_hp[:, i, d0:w], p_hp[:, i, d0:w],
                            cmask_sb[:, k, 0:w - d0])
                if m_w > tlo:
                    for i, h in enumerate(heads):
                        # span mask precomputed on host: one 2x-mode multiply
                        nc.vector.tensor_mul(
                            p_hp[:, i, tlo:m_w], p_hp[:, i, tlo:m_w],
                            smask_sb[:, h, moff:moff + m_w - tlo])
                # software pipeline: attnV for the PREVIOUS block runs now,
                # so it never waits on this block's exp/mask chain
                if pending is not None:
                    emit_av(*pending)
                pending = (st, tlo, w, p_hp)
            emit_av(*pending)
            # rows 0:64 numerator (scaled SW); rows 64:128 denominator W
            nchunk = TCH // renorm_tts
            for rchunk in range(renorm_tts):
                cs = slice(rchunk * nchunk, (rchunk + 1) * nchunk)
                for i, h in enumerate(heads):
                    hp = (h % 2) * 64
                    pout = pouts[i]
                    rw = scr.tile([DH, TCH], F32, tag="rw", bufs=4,
                                  name=f"rw{h}_{rchunk}")
                    with nc.allow_low_precision(reason="denom recip"):
                        nc.vector.reciprocal(rw[:, cs], pout[DH:2 * DH, cs])
                    nc.vector.scalar_tensor_tensor(
                        out_pair[j][tch][hp:hp + DH, cs], pout[0:DH, cs],
                        1.0 / SW, rw[:, cs],
                        op0=mybir.AluOpType.mult, op1=mybir.AluOpType.mult)

        def op_matmuls(yps, tch, toff, ns):
            for j in range(4):
                nc.tensor.matmul(
                    yps[:], out_pair[j][tch][:, toff:toff + 128],
                    wo_sb[:, j, ns],
                    start=(j == 0), stop=(j == 3), skip_group_check=True)

        def yo_copy(yo, ns, yps, eng):
            with nc.allow_low_precision(reason="f16 output"):
                if eng % 2 == 0:
                    nc.scalar.copy(yo[:, ns], yps[:])
                else:
                    nc.vector.tensor_copy(yo[:, ns], yps[:])

        def op_tt(tt, split_dma=False, pool=None):
            """Out-projection for t-tile tt: both 512-chunks + DMA out."""
            pool = pool or ps_proj
            tch = tt // 4
            toff = 128 * tt - TCH * tch
            yo = ysb.tile([128, D], F16, tag="y")
            for nch in range(NT):
                ns = slice(TCH * nch, TCH * (nch + 1))
                yps = pool.tile([128, TCH], F32, tag="pj",
                                name=f"y{tt}_{nch}")
                op_matmuls(yps, tch, toff, ns)
                # the final tiles' copies go on the fast engines (ACT/DVE)
                # and issue their DMA from the ACT queue (skips the busy
                # SP queue at the very end of the kernel)
                eng = nch if tt >= 6 else (tt + nch) % 3
                yo_copy(yo, ns, yps, eng)
                if split_dma:
                    deng = nc.scalar if eng == 0 else nc.sync
                    deng.dma_start(yp_d[128 * tt:128 * (tt + 1), ns],
                                   yo[:, ns])
            if not split_dma:
                nc.sync.dma_start(yp_d[128 * tt:128 * (tt + 1), :], yo[:])

        qk_rest = [(qt_sb, "wq", 2, 0), (kt_sb, "wk", 2, 0),
                   (qt_sb, "wq", 3, 0), (kt_sb, "wk", 3, 0),
                   (qt_sb, "wq", 2, 1), (kt_sb, "wk", 2, 1),
                   (qt_sb, "wq", 3, 1), (kt_sb, "wk", 3, 1)]

        def qk_f(idx):
            dst, wn, et, tch = qk_rest[idx]
            return lambda: emit_qk(dst, wn, et, tch, ps_proj)

        attn_pair(0, 0, v_prefetch=True,
                  fillers=[qk_f(0), None, qk_f(1), None])
        attn_pair(0, 1, fillers=[qk_f(2), qk_f(3)])
        attn_pair(0, 2, fillers=[qk_f(4), qk_f(5)])
        attn_pair(0, 3, fillers=[qk_f(6), qk_f(7)])
        def op_nch(tt, nch):
            """Half of op_tt as a filler unit; DMA fires on the second half."""
            tch = tt // 4
            toff = 128 * tt - TCH * tch
            ns = slice(TCH * nch, TCH * (nch + 1))
            if nch == 0:
                op_nch.yo[tt] = ysb.tile([128, D], F16, tag="y",
                                         name=f"yon{tt}")
            yo = op_nch.yo[tt]
            yps = ps_proj.tile([128, TCH], F32, tag="pj", name=f"y{tt}_{nch}")
            op_matmuls(yps, tch, toff, ns)
            yo_copy(yo, ns, yps, (tt + nch) % 3)
            if nch == NT - 1:
                nc.sync.dma_start(yp_d[128 * tt:128 * (tt + 1), :], yo[:])
        op_nch.yo = {}

        attn_pair(1, 0, fillers=[lambda: op_nch(0, 0), lambda: op_nch(0, 1)])
        attn_pair(1, 1, fillers=[lambda: op_nch(1, 0), lambda: op_nch(1, 1)])
        attn_pair(1, 2, fillers=[lambda: op_nch(2, 0), lambda: op_nch(2, 1)])
        attn_pair(1, 3, fillers=[lambda: op_nch(3, 0), None,
                                 lambda: dummy(3, ap=256)], renorm_tts=4)
        op_nch(3, 1)
        # attention PSUM pools are done; hand their banks to the tail
        # out-projections so four groups can be in flight
        attn_ctx.close()
        ps_tail = ctx.enter_context(
            tc.tile_pool(name="ps_tail", bufs=4, space="PSUM"))
        for tt in range(4, 7):
            op_tt(tt, pool=ps_tail)
        op_tt(7, split_dma=True, pool=ps_tail)

    nc.compile()
    _NC_CACHE[key] = nc
    return nc


def _f8pair(a):
    f8 = ml_dtypes.float8_e4m3fn
    hi = a.astype(f8)
    lo = (a - hi.astype(np.float32)).astype(f8)
    return hi, lo


def _pm_pairtiles(a):
    """[Ktot, N] -> partition-major [128, Ktot/256, 2, N]:
    out[p, m, i, :] = a[256m + 128i + p, :]."""
    K = a.shape[0]
    rest = a.shape[1:]
    return np.ascontiguousarray(
        a.reshape(K // 256, 2, 128, *rest).transpose(2, 0, 1, 3))


def _prep_core_inputs(x, Wq, Wk, Wv, Wo, Wspan, bspan, span_full, cmask):
    bf = ml_dtypes.bfloat16
    in_maps = []
    # span net on host (f32): z per (batch, head)
    logits = x.mean(axis=1) @ Wspan.T + bspan
    z = T / (1.0 + np.exp(-logits))
    for c in range(N_CORES):
        b, g = c // 2, c % 2
        hs = slice(E * g, E * (g + 1))
        xt = np.ascontiguousarray(x[b].T).astype(np.float32)
        xh, xl = _f8pair(xt)
        xhp = _pm_pairtiles(xh)
        xlp = _pm_pairtiles(xl)
        m = {
            "cmask": cmask,
            "xh0": np.ascontiguousarray(xhp[:, 0:2]),
            "xh1": np.ascontiguousarray(xhp[:, 2:4]),
            "xl0": np.ascontiguousarray(xlp[:, 0:2]),
            "xl1": np.ascontiguousarray(xlp[:, 2:4]),
            "smask": _make_smask(z[b, HC * g:HC * (g + 1)], span_full),
        }
        m["woT"] = np.ascontiguousarray(
            Wo[:, hs].T.reshape(4, 128, D).transpose(1, 0, 2)).astype(bf)
        for wname, W in (("wq", Wq), ("wk", Wk), ("wv", Wv)):
            wt = np.ascontiguousarray(W[hs, :].T).astype(np.float32) * SW
            wh, wl = _f8pair(wt)
            m[wname + "h"] = _pm_pairtiles(wh)
            m[wname + "l"] = _pm_pairtiles(wl)
        in_maps.append(m)
    return in_maps


def _make_c01():
    """cmask: [:, 0:4] causal 0/1 (1 where s' >= j), [:, 4] identity."""
    sp = np.arange(128, dtype=np.float32)[:, None]
    jp = np.arange(128, dtype=np.float32)[None, :]
    c01 = (sp - jp >= 0).astype(np.float32)
    ident = np.eye(128, dtype=np.float32)
    stk = np.stack([c01, c01, c01, c01, ident])  # [5, 128, 128]
    return np.ascontiguousarray(stk.transpose(1, 0, 2)).astype(
        ml_dtypes.bfloat16)


def _make_smask(z_heads, span_full):
    """Per-head span masks clip((R + z - d)/R, 0, 1), packed like the
    kernel's block windows: [128 s', HC, total span cols]."""
    sp = np.arange(128, dtype=np.float32)[:, None]
    cols = []
    for tch in range(NT):
        for k in range(ST):
            tlo = t_lo(k, tch, span_full)
            m_w = span_width(k, tch, span_full)
            if m_w - tlo <= 0:
                continue
            delta = 128 * k - 512 * tch
            tp = np.arange(tlo, m_w, dtype=np.float32)[None, :]
            d = delta + sp - tp
            cols.append(np.where(d < 0, 0.0, d))
    if not cols:
        return np.zeros((128, HC, 1), np.float16)
    dall = np.concatenate(cols, axis=1)  # [128, S]
    mask = np.clip((R + z_heads[None, :, None] - dall[:, None, :]) / R,
                   0.0, 1.0)
    return mask.astype(np.float16)


def kernel(x, Wq, Wk, Wv, Wo, bo, Wspan, bspan):
    x = np.asarray(x, np.float32)
    Wq = np.asarray(Wq, np.float32)
    Wk = np.asarray(Wk, np.float32)
    Wv = np.asarray(Wv, np.float32)
    Wo = np.asarray(Wo, np.float32)
    bo = np.asarray(bo, np.float32)
    Wspan = np.asarray(Wspan, np.float32)
    bspan = np.asarray(bspan, np.float32)

    # span-mask restriction is only exact when z in [Z_MIN+6, Z_MAX-6]
    logits = x.mean(axis=1) @ Wspan.T + bspan
    z = T / (1.0 + np.exp(-logits))
    span_full = bool(z.min() < Z_MIN + 6.0 or z.max() > Z_MAX - 6.0)
    nc = build_nc(span_full=span_full)
    in_maps = _prep_core_inputs(x, Wq, Wk, Wv, Wo, Wspan, bspan,
                                span_full, _make_c01())
    res = run_bass_kernel_spmd(nc, in_maps, core_ids=list(range(N_CORES)))
    y = np.empty((B, T, D), np.float32)
    for b in range(B):
        y[b] = (res.results[2 * b]["yp"].astype(np.float32)
                + res.results[2 * b + 1]["yp"].astype(np.float32) + bo)
    return y
